# revision 4
# baseline (speedup 1.0000x reference)
"""Trainium2 Bass kernel for nn_Attention_6794638262338 (v2).

Single-layer attention block with BitNet-style ternary-quantized projections:
    x -> LN1 -> qkv proj (ternary W) -> MHA softmax -> LN2 -> out proj (ternary W)

Strategy: pure data parallelism. batch=8, n_cores=8 -> one batch element per
core, no collectives. Each core runs an identical Bass/Tile program.

v2 redesign vs v1 (cost-model-driven):
  - attn@v is computed row-major: out[n_tile, (head, 65)] accumulated over key
    tiles, with lhsT = atn tile (keys on partitions) and rhs = v rows. PE cost
    is the moving dim only, so this halves attn@v matmul rows vs the old
    [65, n] orientation, and the softmax colsum (ones column in v) lands as a
    per-partition scalar -> the divide is a cheap tensor_scalar, no partition
    broadcast needed.
  - LN2 stats come from bn_stats/bn_aggr on the row-major divided activations
    (per head-pair groups, equal counts -> exact), replacing the transposed
    ones-matmul + gpsimd-squares machinery.
  - a^T for the output projection is built with PE identity transposes.
  - PSUM->SBUF copies are spread across DVE and GpSimd (Pool); pair-3 divides
    run on ACT which is idle after the last exp.
  - Weight DMAs are split per block and interleaved with x so the first
    scores/exp start early.

Math folds (host side), same as v1:
  - ternary_quant(W) = T * s with T in {-1,0,1}: pass T in bf16 (exact), fold
    s_qkv^2 * DIM_HEAD^-0.5 into the exp() activation scale, fold s_qkv/s_out
    into the LN2 rsqrt epsilon/scale.
  - softmax denominator folded: y-scalars divide by colsum via LN2 stats of
    the divided activations; the activation divide itself is a per-partition
    tensor_scalar with the reciprocal of the colsum column.
"""

import numpy as np
from contextlib import ExitStack

import concourse.bass as bass
import concourse.mybir as mybir
import concourse.tile as tile
from concourse import bacc
from concourse.bass import ts, ds
from concourse.bass_utils import run_bass_kernel_spmd
from concourse.masks import make_identity

F32 = mybir.dt.float32
BF16 = mybir.dt.bfloat16
AF = mybir.ActivationFunctionType
ALU = mybir.AluOpType

B, N, D = 8, 1024, 512
H, DH = 8, 64
INNER = H * DH  # 512
NT = N // 128   # 8 n-tiles
DC = D // 128   # 4 d-chunks
NP = H // 2     # 4 head pairs
EPS_LN = 1e-5
EPS_Q = 1e-6

TRACE = False          # set by test.py to capture an NTFF profile
LAST_RESULTS = None    # BassKernelResults of the most recent run

_CACHE = {}
YMUL_POOL = True
# per-pair key-tile sets whose exp runs on DVE / Pool instead of ACT
# (exp(scale*s) == (e**scale)**s: constant-base pow on the vector ALUs)
EXP_DVE_MTS = ()
EXP_POOL_MTS = ()


def _ternary(w):
    """Replicate reference ternary_quant in fp32; return (unit ternary, scale)."""
    w = np.asarray(w, np.float32)
    s = np.float32(np.mean(np.abs(w), dtype=np.float32))
    t = np.round(np.clip(w / (s + np.float32(EPS_Q)), -1.0, 1.0)).astype(np.float32)
    return t, float(s)


def _emit(ctx: ExitStack, tc: "tile.TileContext", io: dict, c: dict, sfx: str = ""):
    nc = tc.nc
    dbg = c.get("debug", False)

    def dump(name, ap):
        if dbg:
            d = nc.dram_tensor(f"dbg_{name}{sfx}", list(ap.shape), ap.dtype, kind="ExternalOutput").ap()
            nc.sync.dma_start(out=d, in_=ap)

    x, tqT, toT, w1u, y = io["x"], io["tqT"], io["toT"], io["w1u"], io["y"]

    need_g1 = c["need_g1"]
    need_b1 = c["need_b1"]
    need_bt = c["need_bt"]

    # ---------------- pools ----------------
    const_p = ctx.enter_context(tc.tile_pool(name="const" + sfx, bufs=1))
    xp = ctx.enter_context(tc.tile_pool(name="xp" + sfx, bufs=8))
    lnp = ctx.enter_context(tc.tile_pool(name="lnp" + sfx, bufs=4))
    xlnp = ctx.enter_context(tc.tile_pool(name="xlnp" + sfx, bufs=5))
    big = ctx.enter_context(tc.tile_pool(name="big" + sfx, bufs=1))
    attp = ctx.enter_context(tc.tile_pool(name="attp" + sfx, bufs=3))
    smp = ctx.enter_context(tc.tile_pool(name="smp" + sfx, bufs=4))
    outp = ctx.enter_context(tc.tile_pool(name="outp" + sfx, bufs=4))
    # PSUM budget: 8 banks = ps_s ([128,1024] x2 = 4) + ps_m ([128,512] x2 = 2)
    #              + ps_av ([128,130] x2 = 2)
    ps_s = ctx.enter_context(tc.tile_pool(name="ps_s" + sfx, bufs=2, space="PSUM"))
    ps_m = ctx.enter_context(tc.tile_pool(name="ps_m" + sfx, bufs=2, space="PSUM"))
    ps_av = ctx.enter_context(tc.tile_pool(name="ps_av" + sfx, bufs=2, space="PSUM"))

    # ---------------- constants (no DMA yet) ----------------
    ident = const_p.tile([128, 128], BF16)
    make_identity(nc, ident)
    eps1 = const_p.tile([128, 1], F32)
    nc.vector.memset(eps1, float(EPS_LN))
    eps2 = const_p.tile([128, 1], F32)
    nc.vector.memset(eps2, c["eps_eff"])
    if EXP_DVE_MTS or EXP_POOL_MTS:
        ebase = const_p.tile([128, N], F32)
        nc.vector.memset(ebase, float(np.exp(c["scale_exp"])))
    # warm the ln/exp activation table before the first rstd
    warm = const_p.tile([128, 1], F32)
    nc.scalar.activation(warm, eps1, AF.Ln, bias=eps1)
    nc.scalar.activation(warm, warm, AF.Exp, scale=-0.5)

    # qkv unit-ternary weights, transposed: [d, 3*inner] -> sbuf [128, DC, 3*inner]
    tq_sb = const_p.tile([128, DC, 3 * INNER], BF16)
    tq_src = tqT.rearrange("(c p) o -> p c o", p=128)
    # out-proj unit weights (g2 folded), transposed: [o, dout] -> [128, DC, dout]
    toT_sb = const_p.tile([128, DC, INNER], BF16)
    # W1 rowsums broadcast across partitions
    w1b = const_p.tile([128, INNER], F32)

    def bcast_dma(dst, src_ap, parts=128):
        nc.gpsimd.dma_start(
            out=dst,
            in_=bass.AP(tensor=src_ap.tensor, offset=src_ap.offset,
                        ap=[[0, parts]] + list(src_ap.ap)),
        )

    if need_g1:
        g1b = const_p.tile([128, D], F32)
    if need_b1:
        b1b = const_p.tile([128, D], F32)
    if need_bt:
        btb = const_p.tile([128, INNER], F32)

    # ---------------- persistent big tensors ----------------
    # xln^T: [d, n] bf16 as [128, DC, N]   (partition = d within chunk)
    xlnT = big.tile([128, DC, N], BF16)
    # q^T, k^T head-major: [o, n] as [128, NP, N] (o = pair*128 + p)
    qT = big.tile([128, NP, N], BF16)
    kT = big.tile([128, NP, N], BF16)
    # v row-major, pair-major with ones columns: [128, mt, pair, 130]
    # (cols 0:64 = even head dims, 64 = ones; 65:129 odd head, 129 = ones)
    v_sb = big.tile([128, NT, NP, 2 * (DH + 1)], BF16)
    nc.vector.memset(v_sb[:, :, :, DH : DH + 1], 1.0)
    nc.vector.memset(v_sb[:, :, :, 2 * DH + 1 : 2 * DH + 2], 1.0)
    # divided attention output, row-major: [128, nchunk, (pair sub d)=512] bf16
    a_sb = big.tile([128, NT, INNER], BF16)
    # a^T for the output projection: [o, n] as [128, DC, N]
    aT = big.tile([128, DC, N], BF16)
    # per-(pair, sub, nchunk) bn_stats triples: [128, nchunk, pair*12+sub*6]
    # (flat so bn_aggr reads all 16 equal-count triples in one 1-D free AP)
    st_all = big.tile([128, NT, NP * 12], F32)
    # per-nchunk LN2 aggregates: mean/var [128, nchunk, 2], r2n/mu staging
    mv_all = big.tile([128, NT, 2], F32)
    r2n_all = big.tile([128, NT], F32)

    # ---------------- weight DMA blocks ----------------
    # All big weight DMAs ride the sync queue: pair-0 q/k blocks first, then
    # the x stream, then the remaining blocks — ordered so nothing contends
    # with x on the DMA engines before xlnT is complete.
    def dma_tq_block(sec, ot):
        lo = sec * INNER + ot * 128
        nc.sync.dma_start(out=tq_sb[:, :, ds(lo, 128)], in_=tq_src[:, :, ds(lo, 128)])

    def dma_tq_v():
        nc.sync.dma_start(
            out=tq_sb[:, :, ds(2 * INNER, INNER)],
            in_=tq_src[:, :, ds(2 * INNER, INNER)],
        )

    # small broadcast DMAs on the gpsimd queue (independent, tiny)
    bcast_dma(w1b, w1u)
    if need_g1:
        bcast_dma(g1b, io["g1v"])
    if need_b1:
        bcast_dma(b1b, io["b1v"])
    if need_bt:
        bcast_dma(btb, io["btv"])

    # ---------------- helpers needed inside Phase A ----------------
    _phase_a = [True]

    def qk_copy(dst, src):
        # PSUM->SBUF copies: GPSIMD has no PSUM port. During phase A ACT is
        # idle so qk copies ride it; in-run ACT is the pacer -> DVE.
        if _phase_a[0]:
            nc.scalar.copy(dst, src)
        else:
            nc.vector.tensor_copy(out=dst, in_=src)

    def emit_qk_half(ot, sec, nn, pool=None):
        """One query-half of one section (q or k) of head-pair ot."""
        dst = qT if sec == 0 else kT
        # early (phase-A) calls borrow the attnv pool's bank (idle until
        # phase C; both tile shapes pad to one PSUM bank) so the phase-A
        # transpose ring in ps_m is not serialized behind qk
        pq = (pool or ps_m).tile([128, 512], F32, name="pq",
                                 tag="po" if pool is not None else "mm")
        for dc in range(DC):
            nc.tensor.matmul(
                pq,
                lhsT=tq_sb[:, dc, ds(sec * INNER + ot * 128, 128)],
                rhs=xlnT[:, dc, ts(nn, 512)],
                start=(dc == 0), stop=(dc == DC - 1),
            )
        qk_copy(dst[:, ot, ts(nn, 512)], pq)

    # ================ Phase A: load x, LN1, transpose ================
    # The LN1 chain (stats -> rstd -> normalize) flow-controls the x stream,
    # so transposes+copies lag 3 tiles behind: the copies (alternating
    # ACT/DVE) never sit between rstd ops on the ACT queue.
    xls = {}

    def emit_ln1(nt):
        xt = xp.tile([128, D], BF16, name="xt", tag="xt")
        nc.sync.dma_start(out=xt, in_=x[ts(nt, 128), :])
        if nt == 2:
            dma_tq_block(0, 0)
            dma_tq_block(1, 0)
        if nt == NT - 1:
            # rest of the weight DMAs, queued on sync strictly after x
            for ot in range(1, NP):
                dma_tq_block(0, ot)
                dma_tq_block(1, ot)
            dma_tq_v()
            nc.sync.dma_start(
                out=toT_sb, in_=toT.rearrange("(c p) o -> p c o", p=128)
            )
        st6 = lnp.tile([128, 6], F32, name="st6", tag="st6")
        nc.vector.bn_stats(st6, xt)
        mv = lnp.tile([128, 2], F32, name="mv", tag="mv")
        nc.vector.bn_aggr(mv, st6)
        # rstd = exp(-0.5*ln(var+eps)) — keeps ACT on the ln/exp table set
        sd = lnp.tile([128, 1], F32, name="sd", tag="sd")
        nc.scalar.activation(sd, mv[:, 1:2], AF.Ln, bias=eps1)
        rs = lnp.tile([128, 1], F32, name="rs", tag="rs")
        nc.scalar.activation(rs, sd, AF.Exp, scale=-0.5)
        xl = xlnp.tile([128, D], BF16, name="xl", tag="xl")
        if need_g1 or need_b1:
            xlf = xlnp.tile([128, D], F32, name="xlf", tag="xlf")
            nc.vector.tensor_scalar(
                out=xlf, in0=xt, scalar1=mv[:, 0:1], scalar2=rs,
                op0=ALU.subtract, op1=ALU.mult,
            )
            if need_g1:
                nc.vector.tensor_mul(xlf, xlf, g1b)
            if need_b1:
                nc.vector.tensor_add(xlf, xlf, b1b)
            nc.vector.tensor_copy(xl, xlf)
        else:
            nc.vector.tensor_scalar(
                out=xl, in0=xt, scalar1=mv[:, 0:1], scalar2=rs,
                op0=ALU.subtract, op1=ALU.mult,
            )
        xls[nt] = xl

    def emit_trcopy(nt):
        pt = ps_m.tile([128, DC, 128], F32, name="pt", tag="mm")
        for dc in range(DC):
            nc.tensor.matmul(
                pt[:, dc, :], lhsT=xls[nt][:, ts(dc, 128)], rhs=ident,
                start=True, stop=True,
            )
        nc.scalar.copy(xlnT[:, :, ts(nt, 128)], pt)

    LAG = 3
    for nt in range(NT):
        emit_ln1(nt)
        if nt >= LAG:
            emit_trcopy(nt - LAG)
            if nt - LAG == 3:
                # first query-half of pair-0 q/k (needs xlnT tiles 0..3 only)
                emit_qk_half(0, 0, 0, pool=ps_av)
                emit_qk_half(0, 1, 0, pool=ps_av)
    for nt in range(NT - LAG, NT):
        emit_trcopy(nt)
    emit_qk_half(0, 0, 1, pool=ps_av)
    emit_qk_half(0, 1, 1, pool=ps_av)
    _phase_a[0] = False

    dump("xlnT", xlnT)

    # ================ Phase B helpers ================
    def emit_qk_part(ot, sec):
        """One section (q or k) of head-pair ot: 2 psums of [128, 512]."""
        emit_qk_half(ot, sec, 0)
        emit_qk_half(ot, sec, 1)

    def emit_v(mt):
        # v row-major: psum[m_tile, o] = sum_dc xlnT[:,dc,mt].T @ Tq_v[:,dc,:]
        pv = ps_m.tile([128, 512], F32, name="pv", tag="mm")
        for dc in range(DC):
            nc.tensor.matmul(
                pv,
                lhsT=xlnT[:, dc, ts(mt, 128)],
                rhs=tq_sb[:, dc, ds(2 * INNER, INNER)],
                start=(dc == 0), stop=(dc == DC - 1),
            )
        # strided copy into pair-major layout [pair, sub, 64] with 65-stride
        vv = v_sb[:, mt].rearrange("p pr (s e) -> p pr s e", s=2)[:, :, :, 0:DH]
        nc.vector.tensor_copy(
            out=vv, in_=pv.rearrange("p (pr s d) -> p pr s d", pr=NP, s=2)
        )

    scale_exp = c["scale_exp"]

    def emit_score_sub(p, mt, sub, atns):
        """Scores+exp for one sub-head of pair p, key tile mt."""
        pss = ps_s.tile([128, N], F32, name="pss", tag="s")
        base = sub * 64
        for nn in range(2):
            nc.tensor.matmul(
                pss[:, ts(nn, 512)],
                lhsT=kT[ds(base, 64), p, ts(mt, 128)],
                rhs=qT[ds(base, 64), p, ts(nn, 512)],
                start=True, stop=True,
            )
        nc.scalar.activation(
            out=atns[sub][:, mt, :], in_=pss, func=AF.Exp, scale=scale_exp
        )

    def emit_attnv_sub(p, j, sub, atns, tail=False):
        """attn@v for one sub-head: psum [128, 65], divide, per-sub stats."""
        po = ps_av.tile([128, DH + 1], F32, name="pos", tag="po")
        for mt in range(NT):
            nc.tensor.matmul(
                po,
                lhsT=atns[sub][:, mt, ts(j, 128)],
                rhs=v_sb[:, mt, p, ds(sub * (DH + 1), DH + 1)],
                start=(mt == 0), stop=(mt == NT - 1),
            )
        rc = smp.tile([128, 1], F32, name="rcs", tag="rcs")
        nc.vector.reciprocal(rc, po[:, DH : DH + 1])
        dst = a_sb[:, j, ds(p * 128 + sub * DH, DH)]
        if tail:
            nc.scalar.activation(dst, po[:, 0:DH], AF.Copy, scale=rc)
        else:
            nc.vector.tensor_scalar_mul(dst, po[:, 0:DH], rc)
        nc.vector.bn_stats(
            st_all[:, j, ds(p * 12 + sub * 6, 6)], dst
        )

    def emit_transpose_sub(p, j, sub, act=False):
        """Transpose one sub-head block of chunk j into aT."""
        pt = ps_m.tile([64, 128], F32, name="pts", tag="mm")
        nc.tensor.matmul(
            pt, lhsT=a_sb[:, j, ds(p * 128 + sub * DH, DH)], rhs=ident,
            start=True, stop=True,
        )
        dst = aT[ds(sub * DH, DH), p, ts(j, 128)]
        if act:
            nc.scalar.copy(dst, pt)
        else:
            nc.vector.tensor_copy(out=dst, in_=pt)

    def emit_score_mt(p, mt, atns):
        """Scores+exp for head pair p, key tile mt."""
        pss = [
            ps_s.tile([128, N], F32, name="pssa", tag="s"),
            ps_s.tile([128, N], F32, name="pssb", tag="s"),
        ]
        for nn in range(2):
            for sub in range(2):
                base = sub * 64
                nc.tensor.matmul(
                    pss[sub][:, ts(nn, 512)],
                    lhsT=kT[ds(base, 64), p, ts(mt, 128)],
                    rhs=qT[ds(base, 64), p, ts(nn, 512)],
                    start=True, stop=True,
                )
        for sub in range(2):
            out = atns[sub][:, mt, :]
            if p < NP - 1 and mt in EXP_DVE_MTS and sub == 0:
                nc.vector.tensor_tensor(out=out, in0=ebase, in1=pss[sub], op=ALU.pow)
            elif p < NP - 1 and mt in EXP_POOL_MTS and sub == 1:
                nc.gpsimd.tensor_tensor(out=out, in0=ebase, in1=pss[sub], op=ALU.pow)
            else:
                nc.scalar.activation(
                    out=out, in_=pss[sub], func=AF.Exp, scale=scale_exp
                )

    def emit_attnv_chunk(p, j, atns, tail=False):
        """attn@v for head pair p, query chunk j -> divided rows of a_sb."""
        po = ps_av.tile([128, 2 * (DH + 1)], F32, name="po", tag="po")
        for sub in range(2):
            for mt in range(NT):
                nc.tensor.matmul(
                    po[:, ds(sub * (DH + 1), DH + 1)],
                    lhsT=atns[sub][:, mt, ts(j, 128)],
                    rhs=v_sb[:, mt, p, ds(sub * (DH + 1), DH + 1)],
                    start=(mt == 0), stop=(mt == NT - 1),
                )
        # reciprocal of the two colsum columns (per-partition scalars)
        rc = smp.tile([128, 2, 1], F32, name="rc", tag="rc")
        cs_view = po.rearrange("q (s e) -> q s e", s=2)[:, :, DH : DH + 1]
        nc.vector.reciprocal(rc, cs_view)
        # divide + downcast into row-major a_sb; tail pairs use ACT (idle
        # after the last exp), steady-state pairs split DVE/Pool
        for sub in range(2):
            dst = a_sb[:, j, ds(p * 128 + sub * DH, DH)]
            src = po[:, ds(sub * (DH + 1), DH)]
            if tail:
                nc.scalar.activation(dst, src, AF.Copy, scale=rc[:, sub, :])
            else:
                nc.vector.tensor_scalar_mul(dst, src, rc[:, sub, :])
        # per-pair LN2 stats of this chunk (equal counts -> exact bn_aggr)
        for sub in range(2):
            nc.vector.bn_stats(
                st_all[:, j, ds(p * 12 + sub * 6, 6)],
                a_sb[:, j, ds(p * 128 + sub * DH, DH)],
            )

    def emit_transpose(p, jlo, njs, act=False):
        """Transpose a_sb[:, j, p-block] -> aT[:, p, j*128] for njs chunks."""
        pt = ps_m.tile([128, njs * 128], F32, name="ptr", tag="mm")
        for i in range(njs):
            nc.tensor.matmul(
                pt[:, ts(i, 128)], lhsT=a_sb[:, jlo + i, ds(p * 128, 128)],
                rhs=ident, start=True, stop=True,
            )
        dst = aT[:, p, ds(jlo * 128, njs * 128)]
        if act:
            nc.scalar.copy(dst, pt)
        else:
            nc.vector.tensor_copy(out=dst, in_=pt)

    def emit_ln2_aggr(j):
        nc.vector.bn_aggr(mv_all[:, j, :], st_all[:, j, :])

    def emit_r2_batch(j0, n):
        # r2 = s_o / sqrt(var + eps_eff); r2n = -r2, n chunks at once
        sd2 = smp.tile([128, 4], F32, name="sd2", tag="sd2")
        var_v = mv_all[:, j0 : j0 + n, 1]
        nc.scalar.activation(sd2[:, 0:n], var_v, AF.Ln, bias=eps2, scale=c["inv_so2"])
        r2 = smp.tile([128, 4], F32, name="r2", tag="r2")
        nc.scalar.activation(r2[:, 0:n], sd2[:, 0:n], AF.Exp, scale=-0.5)
        nc.vector.tensor_scalar_mul(r2n_all[:, j0 : j0 + n], r2[:, 0:n], -1.0)

    def emit_zy(j):
        """Output projection + LN2 affine for query chunk j."""
        pz = ps_m.tile([128, INNER], F32, name="pz", tag="mm")
        for ch in range(DC):
            nc.tensor.matmul(
                pz, lhsT=aT[:, ch, ts(j, 128)], rhs=toT_sb[:, ch, :],
                start=(ch == 0), stop=(ch == DC - 1),
            )
        yt = outp.tile([128, INNER], F32, name="yt", tag="yt")
        # u = W1*mu - z  (DVE: reads PSUM);  y = u * (-r2) split ACT/DVE so
        # no single engine paces the tail
        nc.vector.scalar_tensor_tensor(
            out=yt, in0=w1b, scalar=mv_all[:, j, 0:1], in1=pz,
            op0=ALU.mult, op1=ALU.subtract,
        )
        if j >= NT - 2:
            nc.scalar.mul(yt, yt, r2n_all[:, j : j + 1])
        elif YMUL_POOL:
            nc.gpsimd.tensor_scalar_mul(yt, yt, r2n_all[:, j : j + 1])
        else:
            nc.vector.tensor_scalar_mul(yt, yt, r2n_all[:, j : j + 1])
        if need_bt:
            nc.vector.tensor_add(yt, yt, btb)
        nc.sync.dma_start(out=y[ts(j, 128), :], in_=yt)

    # ================ Phase C driver ================
    # Pair p's scores stream per key-tile mt; exp (ACT) is the pacer. PE
    # fill-work (qk of the next pair, v, attn@v + transposes of the previous
    # pair) is emitted BEFORE each score tile so it runs in the stall gaps.
    atn_pairs = []

    def new_atn_tiles():
        return [
            attp.tile([128, NT, N], BF16, name=f"atn{s}", tag=f"atn{s}")
            for s in range(2)
        ]

    # fillers[p] = list of thunks to interleave into pair p's 8 mt slots.
    # attn@v of the previous pair comes FIRST so its atn tiles free early
    # (the attp pool has 2 slots; pair p+1's exps wait on pair p-1's attnv).
    fillers = {p: [] for p in range(NP)}
    fillers[0] = [
        lambda: emit_qk_part(1, 0), lambda: emit_qk_part(1, 1),
    ] + [lambda mt=mt: emit_v(mt) for mt in range(6)]
    fillers[1] = [lambda mt=mt: emit_v(mt) for mt in (6, 7)]
    tail_fill = {
        1: [lambda: emit_qk_part(2, 0), lambda: emit_qk_part(2, 1)],
        2: [lambda: emit_qk_part(3, 0), lambda: emit_qk_part(3, 1)],
    }

    def run_pair(p, atns, fl, scores):
        """Emit the 8 score units of `scores` with fillers spread between."""
        per = (len(fl) + NT - 1) // NT if fl else 0
        fi = 0
        for i, sc in enumerate(scores):
            if not (p == 0 and i == 0):
                take = min(per, len(fl) - fi) if fi < len(fl) else 0
                if i == len(scores) - 1:
                    take = len(fl) - fi
                for _ in range(take):
                    fl[fi]()
                    fi += 1
            sc()

    for p in range(NP):
        atns = new_atn_tiles()
        atn_pairs.append(atns)
        fl = list(fillers[p])
        if p >= 1:
            prev = atn_pairs[p - 1]
            av = [
                (lambda p1=p - 1, j=j, a=prev: emit_attnv_chunk(p1, j, a))
                for j in range(NT)
            ]
            fl = fl + av[:4] + tail_fill.get(p, []) + av[4:] + [
                (lambda p1=p - 1: emit_transpose(p1, 0, 4)),
                (lambda p1=p - 1: emit_transpose(p1, 4, 4)),
            ]
        else:
            fl = fl + tail_fill.get(p, [])
        if p < NP - 1:
            run_pair(p, atns, fl,
                     [lambda mt=mt: emit_score_mt(p, mt, atns) for mt in range(NT)])
        else:
            # last pair: all sub-0 exps first, then sub-1; sub-0's attn@v,
            # divides, stats and transposes overlap sub-1's 8.3us exp window
            run_pair(p, atns, fl,
                     [lambda mt=mt: emit_score_sub(p, mt, 0, atns) for mt in range(NT)])
            fl2 = []
            for j in range(NT):
                fl2.append(lambda j=j: emit_attnv_sub(p, j, 0, atns))
            for j in range(NT):
                fl2.append(lambda j=j: emit_transpose_sub(p, j, 0))
            run_pair(p, atns, fl2,
                     [lambda mt=mt: emit_score_sub(p, mt, 1, atns) for mt in range(NT)])

    # ================ tail: last pair's attn@v + LN2 + projection ================
    last = atn_pairs[NP - 1]
    for j in range(NT):
        emit_attnv_sub(NP - 1, j, 1, last, tail=True)
        if j >= 2:
            emit_ln2_aggr(j - 2)
            emit_r2_batch(j - 2, 1)
            emit_transpose_sub(NP - 1, j - 2, 1, act=True)
    for j in (NT - 2, NT - 1):
        emit_ln2_aggr(j)
        emit_r2_batch(j, 1)
        emit_transpose_sub(NP - 1, j, 1, act=True)
    for j in range(NT):
        emit_zy(j)

    dump("qT", qT)
    dump("kT", kT)
    dump("v", v_sb)
    dump("a_sb", a_sb)
    dump("aT", aT)


def _build(c: dict):
    nc = bacc.Bacc("TRN2", target_bir_lowering=False, debug=False, num_devices=B)
    io = {
        "x": nc.dram_tensor("x", [N, D], BF16, kind="ExternalInput").ap(),
        "tqT": nc.dram_tensor("tqT", [D, 3 * INNER], BF16, kind="ExternalInput").ap(),
        "toT": nc.dram_tensor("toT", [INNER, INNER], BF16, kind="ExternalInput").ap(),
        "w1u": nc.dram_tensor("w1u", [INNER], F32, kind="ExternalInput").ap(),
        "y": nc.dram_tensor("y", [N, D], F32, kind="ExternalOutput").ap(),
    }
    if c["need_g1"]:
        io["g1v"] = nc.dram_tensor("g1v", [D], F32, kind="ExternalInput").ap()
    if c["need_b1"]:
        io["b1v"] = nc.dram_tensor("b1v", [D], F32, kind="ExternalInput").ap()
    if c["need_bt"]:
        io["btv"] = nc.dram_tensor("btv", [INNER], F32, kind="ExternalInput").ap()
    reps = c.get("body_reps", 1)
    with tile.TileContext(nc) as tc:
        for r in range(reps):
            with ExitStack() as ctx:
                _emit(ctx, tc, io, c, sfx="" if r == 0 else f"_r{r}")

    nc.compile()

    # The act-table-load pass greedily picks the first set containing each
    # function, thrashing between `natural_log` (Ln) and `exp_and_others`
    # (Exp). All activation funcs this kernel uses (Ln, Exp, Copy, Identity)
    # live together in `natural_log_exp_and_others`, so rewrite the first
    # load to that set and drop the rest.
    from concourse.hw_specs import get_activation_tables
    tset = list(get_activation_tables(nc.m.arch).keys())
    nle = tset.index("natural_log_exp_and_others")
    for blk in nc.main_func.blocks:
        keep, first = [], False
        for inst in blk.instructions:
            if type(inst).__name__ == "InstLoadActFuncSet":
                si = getattr(inst, "sync_info", None)
                clean = si is None or (not si.on_wait and not si.on_update)
                if not first:
                    inst.act_func_set_id = nle
                    first = True
                    keep.append(inst)
                elif not clean:
                    inst.act_func_set_id = nle
                    keep.append(inst)
            else:
                keep.append(inst)
        blk.instructions[:] = keep
    return nc


def _prep(inputs):
    g1 = np.asarray(inputs["g1"], np.float32)
    b1 = np.asarray(inputs["b1"], np.float32)
    g2 = np.asarray(inputs["g2"], np.float32)
    b2 = np.asarray(inputs["b2"], np.float32)
    b_out = np.asarray(inputs["b_out"], np.float32)

    Tq, s_q = _ternary(inputs["W_qkv"])   # [3*inner, d]
    To, s_o = _ternary(inputs["W_out"])   # [dout, o]

    Wp = To * g2[None, :]                 # fold g2 (exact when g2 == 1)
    toT = np.ascontiguousarray(Wp.T)      # [o, dout]
    w1u = Wp.sum(axis=1).astype(np.float32)
    bias_total = (b2 @ To.T) * np.float32(s_o) + b_out

    c = {
        "scale_exp": float(s_q * s_q * (DH ** -0.5)),
        "inv_so2": float(1.0 / (s_o * s_o)),
        "eps_eff": float(EPS_LN / (s_q * s_q * s_o * s_o)),
        "need_g1": bool(not np.allclose(g1, 1.0)),
        "need_b1": bool(np.any(b1)),
        "need_bt": bool(np.any(bias_total)),
    }
    arrs = {
        "tqT": np.ascontiguousarray(Tq.T),
        "toT": toT,
        "w1u": w1u,
        "g1": g1, "b1": b1, "bt": bias_total,
    }
    return c, arrs


def _to_bf16(a):
    import ml_dtypes
    return np.asarray(a, np.float32).astype(ml_dtypes.bfloat16)


def kernel(**inputs) -> np.ndarray:
    global LAST_RESULTS
    x = np.asarray(inputs["x"], np.float32)
    assert x.shape == (B, N, D)
    c, arrs = _prep(inputs)

    key = tuple(sorted(c.items()))
    if key not in _CACHE:
        _CACHE[key] = _build(c)
    nc = _CACHE[key]

    base = {
        "tqT": _to_bf16(arrs["tqT"]),
        "toT": _to_bf16(arrs["toT"]),
        "w1u": arrs["w1u"].astype(np.float32),
    }
    if c["need_g1"]:
        base["g1v"] = arrs["g1"]
    if c["need_b1"]:
        base["b1v"] = arrs["b1"]
    if c["need_bt"]:
        base["btv"] = arrs["bt"].astype(np.float32)

    in_maps = [dict(base, x=_to_bf16(np.ascontiguousarray(x[i]))) for i in range(B)]
    res = run_bass_kernel_spmd(nc, in_maps, core_ids=list(range(B)), trace=TRACE)
    LAST_RESULTS = res
    out = np.stack([res.results[i]["y"] for i in range(B)], axis=0)
    return out.astype(np.float32)


def bench_exec_ns(inputs, iters=32, reps=5, body_reps=1):
    """Measure per-execution NEFF time by chaining `iters` sequential
    executions inside one jitted program (chained through the output
    buffers) and comparing against a 1-execution program."""
    import time as _time
    import jax
    from jax.experimental.shard_map import shard_map
    from jax.sharding import Mesh, PartitionSpec, NamedSharding
    from concourse import bass2jax, mybir as _mybir

    x = np.asarray(inputs["x"], np.float32)
    c, arrs = _prep(inputs)
    if body_reps != 1:
        c["body_reps"] = body_reps
    key = tuple(sorted(c.items()))
    if key not in _CACHE:
        _CACHE[key] = _build(c)
    nc = _CACHE[key]
    bass2jax.install_neuronx_cc_hook()

    base = {
        "tqT": _to_bf16(arrs["tqT"]),
        "toT": _to_bf16(arrs["toT"]),
        "w1u": arrs["w1u"].astype(np.float32),
    }
    if c["need_g1"]:
        base["g1v"] = arrs["g1"]
    if c["need_b1"]:
        base["b1v"] = arrs["b1"]
    if c["need_bt"]:
        base["btv"] = arrs["bt"].astype(np.float32)
    in_maps = [dict(base, x=_to_bf16(np.ascontiguousarray(x[i]))) for i in range(B)]

    partition_name = nc.partition_id_tensor.name if nc.partition_id_tensor else None
    in_names, out_names, out_avals, zero_outs = [], [], [], []
    for alloc in nc.m.functions[0].allocations:
        if not isinstance(alloc, mybir.MemoryLocationSet):
            continue
        name = alloc.memorylocations[0].name
        if alloc.kind == "ExternalInput":
            if name != partition_name:
                in_names.append(name)
        elif alloc.kind == "ExternalOutput":
            out_names.append(name)
            shape = tuple(alloc.tensor_shape)
            dtype = mybir.dt.np(alloc.dtype)
            out_avals.append(jax.core.ShapedArray(shape, dtype))
            zero_outs.append(np.zeros(shape, dtype))
    n_params = len(in_names)

    bind_names = list(in_names) + list(out_names)
    if partition_name is not None:
        bind_names.append(partition_name)

    def _body(*args):
        operands = list(args)
        pid = [bass2jax.partition_id_tensor()] if partition_name else []
        outs = bass2jax._bass_exec_p.bind(
            *(operands + pid),
            out_avals=tuple(out_avals),
            in_names=tuple(bind_names),
            out_names=tuple(out_names),
            lowering_input_output_aliases=(),
            sim_require_finite=True,
            sim_require_nnan=True,
            nc=nc,
        )
        return tuple(outs)

    devices = jax.devices()[:B]
    mesh = Mesh(np.asarray(devices), ("core",))
    spec = PartitionSpec("core")
    n_out = len(out_names)
    per_core = [[np.asarray(m[nm]) for nm in in_names] for m in in_maps]
    concat_in = [
        np.concatenate([per_core[cc][i] for cc in range(B)], axis=0)
        for i in range(n_params)
    ]
    concat_zeros = [
        np.zeros((B * z.shape[0], *z.shape[1:]), z.dtype) for z in zero_outs
    ]
    dev_args = [
        jax.device_put(a, NamedSharding(mesh, spec)) for a in concat_in + concat_zeros
    ]

    f = jax.jit(
        shard_map(
            _body, mesh=mesh,
            in_specs=(spec,) * (n_params + n_out),
            out_specs=(spec,) * n_out,
            check_rep=False,
        )
    )
    jax.block_until_ready(f(*dev_args))  # compile + warm

    times = {}
    for k in (1, iters):
        best = float("inf")
        for _ in range(reps):
            t0 = _time.perf_counter()
            r = None
            for _ in range(k):
                r = f(*dev_args)  # async dispatch; device executes in-order
            jax.block_until_ready(r)
            best = min(best, _time.perf_counter() - t0)
        times[k] = best
    exec_ns = (times[iters] - times[1]) / (iters - 1) * 1e9
    return exec_ns, times


# revision 5
# speedup vs baseline: 1.0730x; 1.0730x over previous
"""Trainium2 Bass kernel for nn_Attention_6794638262338 (v2).

Single-layer attention block with BitNet-style ternary-quantized projections:
    x -> LN1 -> qkv proj (ternary W) -> MHA softmax -> LN2 -> out proj (ternary W)

Strategy: pure data parallelism. batch=8, n_cores=8 -> one batch element per
core, no collectives. Each core runs an identical Bass/Tile program.

v2 redesign vs v1 (cost-model-driven):
  - attn@v is computed row-major: out[n_tile, (head, 65)] accumulated over key
    tiles, with lhsT = atn tile (keys on partitions) and rhs = v rows. PE cost
    is the moving dim only, so this halves attn@v matmul rows vs the old
    [65, n] orientation, and the softmax colsum (ones column in v) lands as a
    per-partition scalar -> the divide is a cheap tensor_scalar, no partition
    broadcast needed.
  - LN2 stats come from bn_stats/bn_aggr on the row-major divided activations
    (per head-pair groups, equal counts -> exact), replacing the transposed
    ones-matmul + gpsimd-squares machinery.
  - a^T for the output projection is built with PE identity transposes.
  - PSUM->SBUF copies are spread across DVE and GpSimd (Pool); pair-3 divides
    run on ACT which is idle after the last exp.
  - Weight DMAs are split per block and interleaved with x so the first
    scores/exp start early.

Math folds (host side), same as v1:
  - ternary_quant(W) = T * s with T in {-1,0,1}: pass T in bf16 (exact), fold
    s_qkv^2 * DIM_HEAD^-0.5 into the exp() activation scale, fold s_qkv/s_out
    into the LN2 rsqrt epsilon/scale.
  - softmax denominator folded: y-scalars divide by colsum via LN2 stats of
    the divided activations; the activation divide itself is a per-partition
    tensor_scalar with the reciprocal of the colsum column.
"""

import numpy as np
from contextlib import ExitStack

import concourse.bass as bass
import concourse.mybir as mybir
import concourse.tile as tile
from concourse import bacc
from concourse.bass import ts, ds
from concourse.bass_utils import run_bass_kernel_spmd
from concourse.masks import make_identity

F32 = mybir.dt.float32
BF16 = mybir.dt.bfloat16
AF = mybir.ActivationFunctionType
ALU = mybir.AluOpType

B, N, D = 8, 1024, 512
H, DH = 8, 64
INNER = H * DH  # 512
NT = N // 128   # 8 n-tiles
DC = D // 128   # 4 d-chunks
NP = H // 2     # 4 head pairs
EPS_LN = 1e-5
EPS_Q = 1e-6

TRACE = False          # set by test.py to capture an NTFF profile
LAST_RESULTS = None    # BassKernelResults of the most recent run

_CACHE = {}
YMUL_POOL = True
# per-pair key-tile sets whose exp runs on DVE / Pool instead of ACT
# (exp(scale*s) == (e**scale)**s: constant-base pow on the vector ALUs)
EXP_DVE_MTS = ()
EXP_POOL_MTS = ()


def _ternary(w):
    """Replicate reference ternary_quant in fp32; return (unit ternary, scale)."""
    w = np.asarray(w, np.float32)
    s = np.float32(np.mean(np.abs(w), dtype=np.float32))
    t = np.round(np.clip(w / (s + np.float32(EPS_Q)), -1.0, 1.0)).astype(np.float32)
    return t, float(s)


def _emit(ctx: ExitStack, tc: "tile.TileContext", io: dict, c: dict, sfx: str = ""):
    nc = tc.nc
    dbg = c.get("debug", False)

    def dump(name, ap):
        if dbg:
            d = nc.dram_tensor(f"dbg_{name}{sfx}", list(ap.shape), ap.dtype, kind="ExternalOutput").ap()
            nc.sync.dma_start(out=d, in_=ap)

    x, tqT, toT, w1u, y = io["x"], io["tqT"], io["toT"], io["w1u"], io["y"]

    need_g1 = c["need_g1"]
    need_b1 = c["need_b1"]
    need_bt = c["need_bt"]

    # ---------------- pools ----------------
    const_p = ctx.enter_context(tc.tile_pool(name="const" + sfx, bufs=1))
    xp = ctx.enter_context(tc.tile_pool(name="xp" + sfx, bufs=8))
    lnp = ctx.enter_context(tc.tile_pool(name="lnp" + sfx, bufs=4))
    xlnp = ctx.enter_context(tc.tile_pool(name="xlnp" + sfx, bufs=5))
    big = ctx.enter_context(tc.tile_pool(name="big" + sfx, bufs=1))
    attp = ctx.enter_context(tc.tile_pool(name="attp" + sfx, bufs=3))
    smp = ctx.enter_context(tc.tile_pool(name="smp" + sfx, bufs=4))
    outp = ctx.enter_context(tc.tile_pool(name="outp" + sfx, bufs=4))
    # PSUM budget: 8 banks = ps_s ([128,1024] x2 = 4) + ps_m ([128,512] x2 = 2)
    #              + ps_av ([128,130] x2 = 2)
    ps_s = ctx.enter_context(tc.tile_pool(name="ps_s" + sfx, bufs=2, space="PSUM"))
    ps_m = ctx.enter_context(tc.tile_pool(name="ps_m" + sfx, bufs=2, space="PSUM"))
    ps_av = ctx.enter_context(tc.tile_pool(name="ps_av" + sfx, bufs=2, space="PSUM"))

    # ---------------- constants (no DMA yet) ----------------
    ident = const_p.tile([128, 128], BF16)
    make_identity(nc, ident)
    eps1 = const_p.tile([128, 1], F32)
    nc.vector.memset(eps1, float(EPS_LN))
    eps2 = const_p.tile([128, 1], F32)
    nc.vector.memset(eps2, c["eps_eff"])
    if EXP_DVE_MTS or EXP_POOL_MTS:
        ebase = const_p.tile([128, N], F32)
        nc.vector.memset(ebase, float(np.exp(c["scale_exp"])))
    # warm the ln/exp activation table before the first rstd
    warm = const_p.tile([128, 1], F32)
    nc.scalar.activation(warm, eps1, AF.Ln, bias=eps1)
    nc.scalar.activation(warm, warm, AF.Exp, scale=-0.5)

    # qkv unit-ternary weights, transposed: [d, 3*inner] -> sbuf [128, DC, 3*inner]
    tq_sb = const_p.tile([128, DC, 3 * INNER], BF16)
    tq_src = tqT.rearrange("(c p) o -> p c o", p=128)
    # out-proj unit weights (g2 folded), transposed: [o, dout] -> [128, DC, dout]
    toT_sb = const_p.tile([128, DC, INNER], BF16)
    # W1 rowsums broadcast across partitions
    w1b = const_p.tile([128, INNER], F32)

    def bcast_dma(dst, src_ap, parts=128):
        nc.gpsimd.dma_start(
            out=dst,
            in_=bass.AP(tensor=src_ap.tensor, offset=src_ap.offset,
                        ap=[[0, parts]] + list(src_ap.ap)),
        )

    if need_g1:
        g1b = const_p.tile([128, D], F32)
    if need_b1:
        b1b = const_p.tile([128, D], F32)
    if need_bt:
        btb = const_p.tile([128, INNER], F32)

    # ---------------- persistent big tensors ----------------
    # xln^T: [d, n] bf16 as [128, DC, N]   (partition = d within chunk)
    xlnT = big.tile([128, DC, N], BF16)
    # q^T, k^T head-major: [o, n] as [128, NP, N] (o = pair*128 + p)
    qT = big.tile([128, NP, N], BF16)
    kT = big.tile([128, NP, N], BF16)
    # v row-major, pair-major with ones columns: [128, mt, pair, 130]
    # (cols 0:64 = even head dims, 64 = ones; 65:129 odd head, 129 = ones)
    v_sb = big.tile([128, NT, NP, 2 * (DH + 1)], BF16)
    nc.vector.memset(v_sb[:, :, :, DH : DH + 1], 1.0)
    nc.vector.memset(v_sb[:, :, :, 2 * DH + 1 : 2 * DH + 2], 1.0)
    # divided attention output, row-major: [128, nchunk, (pair sub d)=512] bf16
    a_sb = big.tile([128, NT, INNER], BF16)
    # a^T for the output projection: [o, n] as [128, DC, N]
    aT = big.tile([128, DC, N], BF16)
    # per-(pair, sub, nchunk) bn_stats triples: [128, nchunk, pair*12+sub*6]
    # (flat so bn_aggr reads all 16 equal-count triples in one 1-D free AP)
    st_all = big.tile([128, NT, NP * 12], F32)
    # per-nchunk LN2 aggregates: mean/var [128, nchunk, 2], r2n/mu staging
    mv_all = big.tile([128, NT, 2], F32)
    r2n_all = big.tile([128, NT], F32)

    # ---------------- weight DMA blocks ----------------
    # All big weight DMAs ride the sync queue: pair-0 q/k blocks first, then
    # the x stream, then the remaining blocks — ordered so nothing contends
    # with x on the DMA engines before xlnT is complete.
    def dma_tq_block(sec, ot):
        lo = sec * INNER + ot * 128
        nc.sync.dma_start(out=tq_sb[:, :, ds(lo, 128)], in_=tq_src[:, :, ds(lo, 128)])

    def dma_tq_v():
        nc.sync.dma_start(
            out=tq_sb[:, :, ds(2 * INNER, INNER)],
            in_=tq_src[:, :, ds(2 * INNER, INNER)],
        )

    # small broadcast DMAs on the gpsimd queue (independent, tiny)
    bcast_dma(w1b, w1u)
    if need_g1:
        bcast_dma(g1b, io["g1v"])
    if need_b1:
        bcast_dma(b1b, io["b1v"])
    if need_bt:
        bcast_dma(btb, io["btv"])

    # ---------------- helpers needed inside Phase A ----------------
    _phase_a = [True]

    def qk_copy(dst, src):
        # PSUM->SBUF copies: GPSIMD has no PSUM port. During phase A ACT is
        # idle so qk copies ride it; in-run ACT is the pacer -> DVE.
        if _phase_a[0]:
            nc.scalar.copy(dst, src)
        else:
            nc.vector.tensor_copy(out=dst, in_=src)

    def emit_qk_half(ot, sec, nn, pool=None):
        """One query-half of one section (q or k) of head-pair ot."""
        dst = qT if sec == 0 else kT
        # early (phase-A) calls borrow the attnv pool's bank (idle until
        # phase C; both tile shapes pad to one PSUM bank) so the phase-A
        # transpose ring in ps_m is not serialized behind qk
        pq = (pool or ps_m).tile([128, 512], F32, name="pq",
                                 tag="po" if pool is not None else "mm")
        for dc in range(DC):
            nc.tensor.matmul(
                pq,
                lhsT=tq_sb[:, dc, ds(sec * INNER + ot * 128, 128)],
                rhs=xlnT[:, dc, ts(nn, 512)],
                start=(dc == 0), stop=(dc == DC - 1),
            )
        qk_copy(dst[:, ot, ts(nn, 512)], pq)

    # ================ Phase A: load x, LN1, transpose ================
    # The LN1 chain (stats -> rstd -> normalize) flow-controls the x stream,
    # so transposes+copies lag 3 tiles behind: the copies (alternating
    # ACT/DVE) never sit between rstd ops on the ACT queue.
    xls = {}

    def emit_ln1(nt):
        xt = xp.tile([128, D], BF16, name="xt", tag="xt")
        nc.sync.dma_start(out=xt, in_=x[ts(nt, 128), :])
        if nt == 2:
            dma_tq_block(0, 0)
            dma_tq_block(1, 0)
        if nt == NT - 1:
            # rest of the weight DMAs, queued on sync strictly after x
            for ot in range(1, NP):
                dma_tq_block(0, ot)
                dma_tq_block(1, ot)
            dma_tq_v()
            nc.sync.dma_start(
                out=toT_sb, in_=toT.rearrange("(c p) o -> p c o", p=128)
            )
        st6 = lnp.tile([128, 6], F32, name="st6", tag="st6")
        nc.vector.bn_stats(st6, xt)
        mv = lnp.tile([128, 2], F32, name="mv", tag="mv")
        nc.vector.bn_aggr(mv, st6)
        # rstd = exp(-0.5*ln(var+eps)) — keeps ACT on the ln/exp table set
        sd = lnp.tile([128, 1], F32, name="sd", tag="sd")
        nc.scalar.activation(sd, mv[:, 1:2], AF.Ln, bias=eps1)
        rs = lnp.tile([128, 1], F32, name="rs", tag="rs")
        nc.scalar.activation(rs, sd, AF.Exp, scale=-0.5)
        xl = xlnp.tile([128, D], BF16, name="xl", tag="xl")
        if need_g1 or need_b1:
            xlf = xlnp.tile([128, D], F32, name="xlf", tag="xlf")
            nc.vector.tensor_scalar(
                out=xlf, in0=xt, scalar1=mv[:, 0:1], scalar2=rs,
                op0=ALU.subtract, op1=ALU.mult,
            )
            if need_g1:
                nc.vector.tensor_mul(xlf, xlf, g1b)
            if need_b1:
                nc.vector.tensor_add(xlf, xlf, b1b)
            nc.vector.tensor_copy(xl, xlf)
        else:
            nc.vector.tensor_scalar(
                out=xl, in0=xt, scalar1=mv[:, 0:1], scalar2=rs,
                op0=ALU.subtract, op1=ALU.mult,
            )
        xls[nt] = xl

    def emit_trcopy(nt):
        pt = ps_m.tile([128, DC, 128], F32, name="pt", tag="mm")
        for dc in range(DC):
            nc.tensor.matmul(
                pt[:, dc, :], lhsT=xls[nt][:, ts(dc, 128)], rhs=ident,
                start=True, stop=True,
            )
        nc.scalar.copy(xlnT[:, :, ts(nt, 128)], pt)

    LAG = 3
    for nt in range(NT):
        emit_ln1(nt)
        if nt >= LAG:
            emit_trcopy(nt - LAG)
            if nt - LAG == 3:
                # first query-half of pair-0 q/k (needs xlnT tiles 0..3 only)
                emit_qk_half(0, 0, 0, pool=ps_av)
                emit_qk_half(0, 1, 0, pool=ps_av)
    for nt in range(NT - LAG, NT):
        emit_trcopy(nt)
    emit_qk_half(0, 0, 1, pool=ps_av)
    emit_qk_half(0, 1, 1, pool=ps_av)
    _phase_a[0] = False

    dump("xlnT", xlnT)

    # ================ Phase B helpers ================
    def emit_qk_part(ot, sec):
        """One section (q or k) of head-pair ot: 2 psums of [128, 512]."""
        emit_qk_half(ot, sec, 0)
        emit_qk_half(ot, sec, 1)

    def emit_v(mt):
        # v row-major: psum[m_tile, o] = sum_dc xlnT[:,dc,mt].T @ Tq_v[:,dc,:]
        pv = ps_m.tile([128, 512], F32, name="pv", tag="mm")
        for dc in range(DC):
            nc.tensor.matmul(
                pv,
                lhsT=xlnT[:, dc, ts(mt, 128)],
                rhs=tq_sb[:, dc, ds(2 * INNER, INNER)],
                start=(dc == 0), stop=(dc == DC - 1),
            )
        # strided copy into pair-major layout [pair, sub, 64] with 65-stride
        vv = v_sb[:, mt].rearrange("p pr (s e) -> p pr s e", s=2)[:, :, :, 0:DH]
        nc.vector.tensor_copy(
            out=vv, in_=pv.rearrange("p (pr s d) -> p pr s d", pr=NP, s=2)
        )

    scale_exp = c["scale_exp"]

    def emit_score_sub(p, mt, sub, atns):
        """Scores+exp for one sub-head of pair p, key tile mt."""
        pss = ps_s.tile([128, N], F32, name="pss", tag="s")
        base = sub * 64
        for nn in range(2):
            nc.tensor.matmul(
                pss[:, ts(nn, 512)],
                lhsT=kT[ds(base, 64), p, ts(mt, 128)],
                rhs=qT[ds(base, 64), p, ts(nn, 512)],
                start=True, stop=True,
            )
        nc.scalar.activation(
            out=atns[sub][:, mt, :], in_=pss, func=AF.Exp, scale=scale_exp
        )

    def emit_attnv_sub(p, j, sub, atns, tail=False):
        """attn@v for one sub-head: psum [128, 65], divide, per-sub stats."""
        po = ps_av.tile([128, DH + 1], F32, name="pos", tag="po")
        for mt in range(NT):
            nc.tensor.matmul(
                po,
                lhsT=atns[sub][:, mt, ts(j, 128)],
                rhs=v_sb[:, mt, p, ds(sub * (DH + 1), DH + 1)],
                start=(mt == 0), stop=(mt == NT - 1),
            )
        rc = smp.tile([128, 1], F32, name="rcs", tag="rcs")
        nc.vector.reciprocal(rc, po[:, DH : DH + 1])
        dst = a_sb[:, j, ds(p * 128 + sub * DH, DH)]
        if tail:
            nc.scalar.activation(dst, po[:, 0:DH], AF.Copy, scale=rc)
        else:
            nc.vector.tensor_scalar_mul(dst, po[:, 0:DH], rc)
        nc.vector.bn_stats(
            st_all[:, j, ds(p * 12 + sub * 6, 6)], dst
        )

    def emit_transpose_sub(p, j, sub, act=False):
        """Transpose one sub-head block of chunk j into aT."""
        pt = ps_m.tile([64, 128], F32, name="pts", tag="mm")
        nc.tensor.matmul(
            pt, lhsT=a_sb[:, j, ds(p * 128 + sub * DH, DH)], rhs=ident,
            start=True, stop=True,
        )
        dst = aT[ds(sub * DH, DH), p, ts(j, 128)]
        if act:
            nc.scalar.copy(dst, pt)
        else:
            nc.vector.tensor_copy(out=dst, in_=pt)

    def emit_score_mt(p, mt, atns):
        """Scores+exp for head pair p, key tile mt."""
        pss = [
            ps_s.tile([128, N], F32, name="pssa", tag="s"),
            ps_s.tile([128, N], F32, name="pssb", tag="s"),
        ]
        for nn in range(2):
            for sub in range(2):
                base = sub * 64
                nc.tensor.matmul(
                    pss[sub][:, ts(nn, 512)],
                    lhsT=kT[ds(base, 64), p, ts(mt, 128)],
                    rhs=qT[ds(base, 64), p, ts(nn, 512)],
                    start=True, stop=True,
                )
        for sub in range(2):
            out = atns[sub][:, mt, :]
            if p < NP - 1 and mt in EXP_DVE_MTS and sub == 0:
                nc.vector.tensor_tensor(out=out, in0=ebase, in1=pss[sub], op=ALU.pow)
            elif p < NP - 1 and mt in EXP_POOL_MTS and sub == 1:
                nc.gpsimd.tensor_tensor(out=out, in0=ebase, in1=pss[sub], op=ALU.pow)
            else:
                nc.scalar.activation(
                    out=out, in_=pss[sub], func=AF.Exp, scale=scale_exp
                )

    def emit_attnv_chunk(p, j, atns, tail=False):
        """attn@v for head pair p, query chunk j -> divided rows of a_sb."""
        po = ps_av.tile([128, 2 * (DH + 1)], F32, name="po", tag="po")
        for sub in range(2):
            for mt in range(NT):
                nc.tensor.matmul(
                    po[:, ds(sub * (DH + 1), DH + 1)],
                    lhsT=atns[sub][:, mt, ts(j, 128)],
                    rhs=v_sb[:, mt, p, ds(sub * (DH + 1), DH + 1)],
                    start=(mt == 0), stop=(mt == NT - 1),
                )
        # reciprocal of the two colsum columns (per-partition scalars)
        rc = smp.tile([128, 2, 1], F32, name="rc", tag="rc")
        cs_view = po.rearrange("q (s e) -> q s e", s=2)[:, :, DH : DH + 1]
        nc.vector.reciprocal(rc, cs_view)
        # divide + downcast into row-major a_sb; tail pairs use ACT (idle
        # after the last exp), steady-state pairs split DVE/Pool
        for sub in range(2):
            dst = a_sb[:, j, ds(p * 128 + sub * DH, DH)]
            src = po[:, ds(sub * (DH + 1), DH)]
            if tail:
                nc.scalar.activation(dst, src, AF.Copy, scale=rc[:, sub, :])
            else:
                nc.vector.tensor_scalar_mul(dst, src, rc[:, sub, :])
        # per-pair LN2 stats of this chunk (equal counts -> exact bn_aggr)
        for sub in range(2):
            nc.vector.bn_stats(
                st_all[:, j, ds(p * 12 + sub * 6, 6)],
                a_sb[:, j, ds(p * 128 + sub * DH, DH)],
            )

    def emit_transpose(p, jlo, njs, act=False):
        """Transpose a_sb[:, j, p-block] -> aT[:, p, j*128] for njs chunks."""
        pt = ps_m.tile([128, njs * 128], F32, name="ptr", tag="mm")
        for i in range(njs):
            nc.tensor.matmul(
                pt[:, ts(i, 128)], lhsT=a_sb[:, jlo + i, ds(p * 128, 128)],
                rhs=ident, start=True, stop=True,
            )
        dst = aT[:, p, ds(jlo * 128, njs * 128)]
        if act:
            nc.scalar.copy(dst, pt)
        else:
            nc.vector.tensor_copy(out=dst, in_=pt)

    def emit_ln2_aggr(j):
        nc.vector.bn_aggr(mv_all[:, j, :], st_all[:, j, :])

    def emit_r2_batch(j0, n):
        # r2 = s_o / sqrt(var + eps_eff); r2n = -r2, n chunks at once
        sd2 = smp.tile([128, 4], F32, name="sd2", tag="sd2")
        var_v = mv_all[:, j0 : j0 + n, 1]
        nc.scalar.activation(sd2[:, 0:n], var_v, AF.Ln, bias=eps2, scale=c["inv_so2"])
        r2 = smp.tile([128, 4], F32, name="r2", tag="r2")
        nc.scalar.activation(r2[:, 0:n], sd2[:, 0:n], AF.Exp, scale=-0.5)
        nc.vector.tensor_scalar_mul(r2n_all[:, j0 : j0 + n], r2[:, 0:n], -1.0)

    def emit_zy(j):
        """Output projection + LN2 affine for query chunk j."""
        pz = ps_m.tile([128, INNER], F32, name="pz", tag="mm")
        for ch in range(DC):
            nc.tensor.matmul(
                pz, lhsT=aT[:, ch, ts(j, 128)], rhs=toT_sb[:, ch, :],
                start=(ch == 0), stop=(ch == DC - 1),
            )
        yt = outp.tile([128, INNER], F32, name="yt", tag="yt")
        # u = W1*mu - z  (DVE: reads PSUM);  y = u * (-r2) split ACT/DVE so
        # no single engine paces the tail
        nc.vector.scalar_tensor_tensor(
            out=yt, in0=w1b, scalar=mv_all[:, j, 0:1], in1=pz,
            op0=ALU.mult, op1=ALU.subtract,
        )
        if j >= NT - 2:
            nc.scalar.mul(yt, yt, r2n_all[:, j : j + 1])
        elif YMUL_POOL:
            nc.gpsimd.tensor_scalar_mul(yt, yt, r2n_all[:, j : j + 1])
        else:
            nc.vector.tensor_scalar_mul(yt, yt, r2n_all[:, j : j + 1])
        if need_bt:
            nc.vector.tensor_add(yt, yt, btb)
        nc.sync.dma_start(out=y[ts(j, 128), :], in_=yt)

    # ================ Phase C driver ================
    # Pair p's scores stream per key-tile mt; exp (ACT) is the pacer. PE
    # fill-work (qk of the next pair, v, attn@v + transposes of the previous
    # pair) is emitted BEFORE each score tile so it runs in the stall gaps.
    atn_pairs = []

    def new_atn_tiles():
        return [
            attp.tile([128, NT, N], BF16, name=f"atn{s}", tag=f"atn{s}")
            for s in range(2)
        ]

    # fillers[p] = list of thunks to interleave into pair p's 8 mt slots.
    # attn@v of the previous pair comes FIRST so its atn tiles free early
    # (the attp pool has 2 slots; pair p+1's exps wait on pair p-1's attnv).
    fillers = {p: [] for p in range(NP)}
    fillers[0] = [
        lambda: emit_qk_part(1, 0), lambda: emit_qk_part(1, 1),
    ] + [lambda mt=mt: emit_v(mt) for mt in range(6)]
    fillers[1] = [lambda mt=mt: emit_v(mt) for mt in (6, 7)]
    tail_fill = {
        1: [lambda: emit_qk_part(2, 0), lambda: emit_qk_part(2, 1)],
        2: [lambda: emit_qk_part(3, 0), lambda: emit_qk_part(3, 1)],
    }

    def run_pair(p, atns, fl, scores):
        """Emit the 8 score units of `scores` with fillers spread between."""
        per = (len(fl) + NT - 1) // NT if fl else 0
        fi = 0
        for i, sc in enumerate(scores):
            if not (p == 0 and i == 0):
                take = min(per, len(fl) - fi) if fi < len(fl) else 0
                if i == len(scores) - 1:
                    take = len(fl) - fi
                for _ in range(take):
                    fl[fi]()
                    fi += 1
            sc()

    for p in range(NP):
        atns = new_atn_tiles()
        atn_pairs.append(atns)
        fl = list(fillers[p])
        if p >= 1:
            prev = atn_pairs[p - 1]
            av = [
                (lambda p1=p - 1, j=j, a=prev: emit_attnv_chunk(p1, j, a))
                for j in range(NT)
            ]
            fl = fl + av[:4] + tail_fill.get(p, []) + av[4:] + [
                (lambda p1=p - 1: emit_transpose(p1, 0, 4)),
                (lambda p1=p - 1: emit_transpose(p1, 4, 4)),
            ]
        else:
            fl = fl + tail_fill.get(p, [])
        if p < NP - 1:
            run_pair(p, atns, fl,
                     [lambda mt=mt: emit_score_mt(p, mt, atns) for mt in range(NT)])
        else:
            # last pair: all sub-0 exps first, then sub-1; sub-0's attn@v,
            # divides, stats and transposes overlap sub-1's 8.3us exp window
            run_pair(p, atns, fl,
                     [lambda mt=mt: emit_score_sub(p, mt, 0, atns) for mt in range(NT)])
            fl2 = []
            for j in range(NT):
                fl2.append(lambda j=j: emit_attnv_sub(p, j, 0, atns))
            for j in range(NT):
                fl2.append(lambda j=j: emit_transpose_sub(p, j, 0))
            run_pair(p, atns, fl2,
                     [lambda mt=mt: emit_score_sub(p, mt, 1, atns) for mt in range(NT)])

    # ================ tail: last pair's attn@v + LN2 + projection ================
    last = atn_pairs[NP - 1]
    for j in range(NT):
        emit_attnv_sub(NP - 1, j, 1, last, tail=True)
        if j >= 2:
            emit_ln2_aggr(j - 2)
            emit_r2_batch(j - 2, 1)
            emit_transpose_sub(NP - 1, j - 2, 1, act=True)
        if j in (4, 5):
            emit_zy(j - 4)
    for j in (NT - 2, NT - 1):
        emit_ln2_aggr(j)
        emit_r2_batch(j, 1)
        emit_transpose_sub(NP - 1, j, 1, act=True)
    for j in range(2, NT):
        emit_zy(j)

    dump("qT", qT)
    dump("kT", kT)
    dump("v", v_sb)
    dump("a_sb", a_sb)
    dump("aT", aT)


def _build(c: dict):
    nc = bacc.Bacc("TRN2", target_bir_lowering=False, debug=False, num_devices=B)
    io = {
        "x": nc.dram_tensor("x", [N, D], BF16, kind="ExternalInput").ap(),
        "tqT": nc.dram_tensor("tqT", [D, 3 * INNER], BF16, kind="ExternalInput").ap(),
        "toT": nc.dram_tensor("toT", [INNER, INNER], BF16, kind="ExternalInput").ap(),
        "w1u": nc.dram_tensor("w1u", [INNER], F32, kind="ExternalInput").ap(),
        "y": nc.dram_tensor("y", [N, D], F32, kind="ExternalOutput").ap(),
    }
    if c["need_g1"]:
        io["g1v"] = nc.dram_tensor("g1v", [D], F32, kind="ExternalInput").ap()
    if c["need_b1"]:
        io["b1v"] = nc.dram_tensor("b1v", [D], F32, kind="ExternalInput").ap()
    if c["need_bt"]:
        io["btv"] = nc.dram_tensor("btv", [INNER], F32, kind="ExternalInput").ap()
    reps = c.get("body_reps", 1)
    with tile.TileContext(nc) as tc:
        for r in range(reps):
            with ExitStack() as ctx:
                _emit(ctx, tc, io, c, sfx="" if r == 0 else f"_r{r}")

    nc.compile()

    # The act-table-load pass greedily picks the first set containing each
    # function, thrashing between `natural_log` (Ln) and `exp_and_others`
    # (Exp). All activation funcs this kernel uses (Ln, Exp, Copy, Identity)
    # live together in `natural_log_exp_and_others`, so rewrite the first
    # load to that set and drop the rest.
    from concourse.hw_specs import get_activation_tables
    tset = list(get_activation_tables(nc.m.arch).keys())
    nle = tset.index("natural_log_exp_and_others")
    for blk in nc.main_func.blocks:
        keep, first = [], False
        for inst in blk.instructions:
            if type(inst).__name__ == "InstLoadActFuncSet":
                si = getattr(inst, "sync_info", None)
                clean = si is None or (not si.on_wait and not si.on_update)
                if not first:
                    inst.act_func_set_id = nle
                    first = True
                    keep.append(inst)
                elif not clean:
                    inst.act_func_set_id = nle
                    keep.append(inst)
            else:
                keep.append(inst)
        blk.instructions[:] = keep
    return nc


def _prep(inputs):
    g1 = np.asarray(inputs["g1"], np.float32)
    b1 = np.asarray(inputs["b1"], np.float32)
    g2 = np.asarray(inputs["g2"], np.float32)
    b2 = np.asarray(inputs["b2"], np.float32)
    b_out = np.asarray(inputs["b_out"], np.float32)

    Tq, s_q = _ternary(inputs["W_qkv"])   # [3*inner, d]
    To, s_o = _ternary(inputs["W_out"])   # [dout, o]

    Wp = To * g2[None, :]                 # fold g2 (exact when g2 == 1)
    toT = np.ascontiguousarray(Wp.T)      # [o, dout]
    w1u = Wp.sum(axis=1).astype(np.float32)
    bias_total = (b2 @ To.T) * np.float32(s_o) + b_out

    c = {
        "scale_exp": float(s_q * s_q * (DH ** -0.5)),
        "inv_so2": float(1.0 / (s_o * s_o)),
        "eps_eff": float(EPS_LN / (s_q * s_q * s_o * s_o)),
        "need_g1": bool(not np.allclose(g1, 1.0)),
        "need_b1": bool(np.any(b1)),
        "need_bt": bool(np.any(bias_total)),
    }
    arrs = {
        "tqT": np.ascontiguousarray(Tq.T),
        "toT": toT,
        "w1u": w1u,
        "g1": g1, "b1": b1, "bt": bias_total,
    }
    return c, arrs


def _to_bf16(a):
    import ml_dtypes
    return np.asarray(a, np.float32).astype(ml_dtypes.bfloat16)


def kernel(**inputs) -> np.ndarray:
    global LAST_RESULTS
    x = np.asarray(inputs["x"], np.float32)
    assert x.shape == (B, N, D)
    c, arrs = _prep(inputs)

    key = tuple(sorted(c.items()))
    if key not in _CACHE:
        _CACHE[key] = _build(c)
    nc = _CACHE[key]

    base = {
        "tqT": _to_bf16(arrs["tqT"]),
        "toT": _to_bf16(arrs["toT"]),
        "w1u": arrs["w1u"].astype(np.float32),
    }
    if c["need_g1"]:
        base["g1v"] = arrs["g1"]
    if c["need_b1"]:
        base["b1v"] = arrs["b1"]
    if c["need_bt"]:
        base["btv"] = arrs["bt"].astype(np.float32)

    in_maps = [dict(base, x=_to_bf16(np.ascontiguousarray(x[i]))) for i in range(B)]
    res = run_bass_kernel_spmd(nc, in_maps, core_ids=list(range(B)), trace=TRACE)
    LAST_RESULTS = res
    out = np.stack([res.results[i]["y"] for i in range(B)], axis=0)
    return out.astype(np.float32)


def bench_exec_ns(inputs, iters=32, reps=5, body_reps=1):
    """Measure per-execution NEFF time by chaining `iters` sequential
    executions inside one jitted program (chained through the output
    buffers) and comparing against a 1-execution program."""
    import time as _time
    import jax
    from jax.experimental.shard_map import shard_map
    from jax.sharding import Mesh, PartitionSpec, NamedSharding
    from concourse import bass2jax, mybir as _mybir

    x = np.asarray(inputs["x"], np.float32)
    c, arrs = _prep(inputs)
    if body_reps != 1:
        c["body_reps"] = body_reps
    key = tuple(sorted(c.items()))
    if key not in _CACHE:
        _CACHE[key] = _build(c)
    nc = _CACHE[key]
    bass2jax.install_neuronx_cc_hook()

    base = {
        "tqT": _to_bf16(arrs["tqT"]),
        "toT": _to_bf16(arrs["toT"]),
        "w1u": arrs["w1u"].astype(np.float32),
    }
    if c["need_g1"]:
        base["g1v"] = arrs["g1"]
    if c["need_b1"]:
        base["b1v"] = arrs["b1"]
    if c["need_bt"]:
        base["btv"] = arrs["bt"].astype(np.float32)
    in_maps = [dict(base, x=_to_bf16(np.ascontiguousarray(x[i]))) for i in range(B)]

    partition_name = nc.partition_id_tensor.name if nc.partition_id_tensor else None
    in_names, out_names, out_avals, zero_outs = [], [], [], []
    for alloc in nc.m.functions[0].allocations:
        if not isinstance(alloc, mybir.MemoryLocationSet):
            continue
        name = alloc.memorylocations[0].name
        if alloc.kind == "ExternalInput":
            if name != partition_name:
                in_names.append(name)
        elif alloc.kind == "ExternalOutput":
            out_names.append(name)
            shape = tuple(alloc.tensor_shape)
            dtype = mybir.dt.np(alloc.dtype)
            out_avals.append(jax.core.ShapedArray(shape, dtype))
            zero_outs.append(np.zeros(shape, dtype))
    n_params = len(in_names)

    bind_names = list(in_names) + list(out_names)
    if partition_name is not None:
        bind_names.append(partition_name)

    def _body(*args):
        operands = list(args)
        pid = [bass2jax.partition_id_tensor()] if partition_name else []
        outs = bass2jax._bass_exec_p.bind(
            *(operands + pid),
            out_avals=tuple(out_avals),
            in_names=tuple(bind_names),
            out_names=tuple(out_names),
            lowering_input_output_aliases=(),
            sim_require_finite=True,
            sim_require_nnan=True,
            nc=nc,
        )
        return tuple(outs)

    devices = jax.devices()[:B]
    mesh = Mesh(np.asarray(devices), ("core",))
    spec = PartitionSpec("core")
    n_out = len(out_names)
    per_core = [[np.asarray(m[nm]) for nm in in_names] for m in in_maps]
    concat_in = [
        np.concatenate([per_core[cc][i] for cc in range(B)], axis=0)
        for i in range(n_params)
    ]
    concat_zeros = [
        np.zeros((B * z.shape[0], *z.shape[1:]), z.dtype) for z in zero_outs
    ]
    dev_args = [
        jax.device_put(a, NamedSharding(mesh, spec)) for a in concat_in + concat_zeros
    ]

    f = jax.jit(
        shard_map(
            _body, mesh=mesh,
            in_specs=(spec,) * (n_params + n_out),
            out_specs=(spec,) * n_out,
            check_rep=False,
        )
    )
    jax.block_until_ready(f(*dev_args))  # compile + warm

    times = {}
    for k in (1, iters):
        best = float("inf")
        for _ in range(reps):
            t0 = _time.perf_counter()
            r = None
            for _ in range(k):
                r = f(*dev_args)  # async dispatch; device executes in-order
            jax.block_until_ready(r)
            best = min(best, _time.perf_counter() - t0)
        times[k] = best
    exec_ns = (times[iters] - times[1]) / (iters - 1) * 1e9
    return exec_ns, times


# revision 6
# speedup vs baseline: 2.2313x; 2.0794x over previous
"""Trainium2 Bass kernel for nn_Attention_6794638262338 (v2).

Single-layer attention block with BitNet-style ternary-quantized projections:
    x -> LN1 -> qkv proj (ternary W) -> MHA softmax -> LN2 -> out proj (ternary W)

Strategy: pure data parallelism. batch=8, n_cores=8 -> one batch element per
core, no collectives. Each core runs an identical Bass/Tile program.

v2 redesign vs v1 (cost-model-driven):
  - attn@v is computed row-major: out[n_tile, (head, 65)] accumulated over key
    tiles, with lhsT = atn tile (keys on partitions) and rhs = v rows. PE cost
    is the moving dim only, so this halves attn@v matmul rows vs the old
    [65, n] orientation, and the softmax colsum (ones column in v) lands as a
    per-partition scalar -> the divide is a cheap tensor_scalar, no partition
    broadcast needed.
  - LN2 stats come from bn_stats/bn_aggr on the row-major divided activations
    (per head-pair groups, equal counts -> exact), replacing the transposed
    ones-matmul + gpsimd-squares machinery.
  - a^T for the output projection is built with PE identity transposes.
  - PSUM->SBUF copies are spread across DVE and GpSimd (Pool); pair-3 divides
    run on ACT which is idle after the last exp.
  - Weight DMAs are split per block and interleaved with x so the first
    scores/exp start early.

Math folds (host side), same as v1:
  - ternary_quant(W) = T * s with T in {-1,0,1}: pass T in bf16 (exact), fold
    s_qkv^2 * DIM_HEAD^-0.5 into the exp() activation scale, fold s_qkv/s_out
    into the LN2 rsqrt epsilon/scale.
  - softmax denominator folded: y-scalars divide by colsum via LN2 stats of
    the divided activations; the activation divide itself is a per-partition
    tensor_scalar with the reciprocal of the colsum column.
"""

import numpy as np
from contextlib import ExitStack

import concourse.bass as bass
import concourse.mybir as mybir
import concourse.tile as tile
from concourse import bacc
from concourse.bass import ts, ds
from concourse.bass_utils import run_bass_kernel_spmd
from concourse.masks import make_identity

F32 = mybir.dt.float32
BF16 = mybir.dt.bfloat16
AF = mybir.ActivationFunctionType
ALU = mybir.AluOpType

B, N, D = 8, 1024, 512
H, DH = 8, 64
INNER = H * DH  # 512
NT = N // 128   # 8 n-tiles
DC = D // 128   # 4 d-chunks
NP = H // 2     # 4 head pairs
EPS_LN = 1e-5
EPS_Q = 1e-6

TRACE = False          # set by test.py to capture an NTFF profile
LAST_RESULTS = None    # BassKernelResults of the most recent run

_CACHE = {}
YMUL_POOL = True
# per-pair key-tile sets whose exp runs on DVE / Pool instead of ACT
# (exp(scale*s) == (e**scale)**s: constant-base pow on the vector ALUs)
EXP_DVE_MTS = ()
EXP_POOL_MTS = ()


def _ternary(w):
    """Replicate reference ternary_quant in fp32; return (unit ternary, scale)."""
    w = np.asarray(w, np.float32)
    s = np.float32(np.mean(np.abs(w), dtype=np.float32))
    t = np.round(np.clip(w / (s + np.float32(EPS_Q)), -1.0, 1.0)).astype(np.float32)
    return t, float(s)


def _emit(ctx: ExitStack, tc: "tile.TileContext", io: dict, c: dict, sfx: str = ""):
    nc = tc.nc
    dbg = c.get("debug", False)

    def dump(name, ap):
        if dbg:
            d = nc.dram_tensor(f"dbg_{name}{sfx}", list(ap.shape), ap.dtype, kind="ExternalOutput").ap()
            nc.sync.dma_start(out=d, in_=ap)

    x, tqT, toT, w1u, y = io["x"], io["tqT"], io["toT"], io["w1u"], io["y"]

    need_g1 = c["need_g1"]
    need_b1 = c["need_b1"]
    need_bt = c["need_bt"]

    # ---------------- pools ----------------
    const_p = ctx.enter_context(tc.tile_pool(name="const" + sfx, bufs=1))
    xp = ctx.enter_context(tc.tile_pool(name="xp" + sfx, bufs=8))
    lnp = ctx.enter_context(tc.tile_pool(name="lnp" + sfx, bufs=4))
    xlnp = ctx.enter_context(tc.tile_pool(name="xlnp" + sfx, bufs=5))
    big = ctx.enter_context(tc.tile_pool(name="big" + sfx, bufs=1))
    attp = ctx.enter_context(tc.tile_pool(name="attp" + sfx, bufs=3))
    smp = ctx.enter_context(tc.tile_pool(name="smp" + sfx, bufs=4))
    outp = ctx.enter_context(tc.tile_pool(name="outp" + sfx, bufs=4))
    # PSUM budget: 8 banks = ps_s ([128,1024] x2 = 4) + ps_m ([128,512] x2 = 2)
    #              + ps_av ([128,130] x2 = 2)
    ps_s = ctx.enter_context(tc.tile_pool(name="ps_s" + sfx, bufs=2, space="PSUM"))
    ps_m = ctx.enter_context(tc.tile_pool(name="ps_m" + sfx, bufs=2, space="PSUM"))
    ps_av = ctx.enter_context(tc.tile_pool(name="ps_av" + sfx, bufs=2, space="PSUM"))

    # ---------------- constants (no DMA yet) ----------------
    ident = const_p.tile([128, 128], BF16)
    make_identity(nc, ident)
    eps1 = const_p.tile([128, 1], F32)
    nc.vector.memset(eps1, float(EPS_LN))
    eps2 = const_p.tile([128, 1], F32)
    nc.vector.memset(eps2, c["eps_eff"])
    if EXP_DVE_MTS or EXP_POOL_MTS:
        ebase = const_p.tile([128, N], F32)
        nc.vector.memset(ebase, float(np.exp(c["scale_exp"])))
    # warm the ln/exp activation table before the first rstd
    warm = const_p.tile([128, 1], F32)
    nc.scalar.activation(warm, eps1, AF.Ln, bias=eps1)
    nc.scalar.activation(warm, warm, AF.Exp, scale=-0.5)

    # qkv unit-ternary weights, transposed: [d, 3*inner] -> sbuf [128, DC, 3*inner]
    tq_sb = const_p.tile([128, DC, 3 * INNER], BF16)
    tq_src = tqT.rearrange("(c p) o -> p c o", p=128)
    # out-proj unit weights (g2 folded), transposed: [o, dout] -> [128, DC, dout]
    toT_sb = const_p.tile([128, DC, INNER], BF16)
    # W1 rowsums broadcast across partitions
    w1b = const_p.tile([128, INNER], F32)

    def bcast_dma(dst, src_ap, parts=128):
        nc.gpsimd.dma_start(
            out=dst,
            in_=bass.AP(tensor=src_ap.tensor, offset=src_ap.offset,
                        ap=[[0, parts]] + list(src_ap.ap)),
        )

    if need_g1:
        g1b = const_p.tile([128, D], F32)
    if need_b1:
        b1b = const_p.tile([128, D], F32)
    if need_bt:
        btb = const_p.tile([128, INNER], F32)

    # ---------------- persistent big tensors ----------------
    # xln^T: [d, n] bf16 as [128, DC, N]   (partition = d within chunk)
    xlnT = big.tile([128, DC, N], BF16)
    # q^T, k^T head-major: [o, n] as [128, NP, N] (o = pair*128 + p)
    qT = big.tile([128, NP, N], BF16)
    kT = big.tile([128, NP, N], BF16)
    # v row-major, pair-major with ones columns: [128, mt, pair, 130]
    # (cols 0:64 = even head dims, 64 = ones; 65:129 odd head, 129 = ones)
    v_sb = big.tile([128, NT, NP, 2 * (DH + 1)], BF16)
    nc.vector.memset(v_sb[:, :, :, DH : DH + 1], 1.0)
    nc.vector.memset(v_sb[:, :, :, 2 * DH + 1 : 2 * DH + 2], 1.0)
    # divided attention output, row-major: [128, nchunk, (pair sub d)=512] bf16
    a_sb = big.tile([128, NT, INNER], BF16)
    # a^T for the output projection: [o, n] as [128, DC, N]
    aT = big.tile([128, DC, N], BF16)
    # per-(pair, sub, nchunk) bn_stats triples: [128, nchunk, pair*12+sub*6]
    # (flat so bn_aggr reads all 16 equal-count triples in one 1-D free AP)
    st_all = big.tile([128, NT, NP * 12], F32)
    # per-nchunk LN2 aggregates: mean/var [128, nchunk, 2], r2n/mu staging
    mv_all = big.tile([128, NT, 2], F32)
    r2n_all = big.tile([128, NT], F32)

    # ---------------- weight DMA blocks ----------------
    # All big weight DMAs ride the sync queue: pair-0 q/k blocks first, then
    # the x stream, then the remaining blocks — ordered so nothing contends
    # with x on the DMA engines before xlnT is complete.
    def dma_tq_block(sec, ot):
        lo = sec * INNER + ot * 128
        nc.sync.dma_start(out=tq_sb[:, :, ds(lo, 128)], in_=tq_src[:, :, ds(lo, 128)])

    def dma_tq_v():
        nc.sync.dma_start(
            out=tq_sb[:, :, ds(2 * INNER, INNER)],
            in_=tq_src[:, :, ds(2 * INNER, INNER)],
        )

    # small broadcast DMAs on the gpsimd queue (independent, tiny)
    bcast_dma(w1b, w1u)
    if need_g1:
        bcast_dma(g1b, io["g1v"])
    if need_b1:
        bcast_dma(b1b, io["b1v"])
    if need_bt:
        bcast_dma(btb, io["btv"])

    # ---------------- helpers needed inside Phase A ----------------
    _phase_a = [True]

    def qk_copy(dst, src):
        # PSUM->SBUF copies: GPSIMD has no PSUM port. During phase A ACT is
        # idle so qk copies ride it; in-run ACT is the pacer -> DVE.
        if _phase_a[0]:
            nc.scalar.copy(dst, src)
        else:
            nc.vector.tensor_copy(out=dst, in_=src)

    def emit_qk_half(ot, sec, nn, pool=None):
        """One query-half of one section (q or k) of head-pair ot."""
        dst = qT if sec == 0 else kT
        # early (phase-A) calls borrow the attnv pool's bank (idle until
        # phase C; both tile shapes pad to one PSUM bank) so the phase-A
        # transpose ring in ps_m is not serialized behind qk
        pq = (pool or ps_m).tile([128, 512], F32, name="pq",
                                 tag="po" if pool is not None else "mm")
        for dc in range(DC):
            nc.tensor.matmul(
                pq,
                lhsT=tq_sb[:, dc, ds(sec * INNER + ot * 128, 128)],
                rhs=xlnT[:, dc, ts(nn, 512)],
                start=(dc == 0), stop=(dc == DC - 1),
            )
        qk_copy(dst[:, ot, ts(nn, 512)], pq)

    # ================ Phase A: load x, LN1, transpose ================
    # The LN1 chain (stats -> rstd -> normalize) flow-controls the x stream,
    # so transposes+copies lag 3 tiles behind: the copies (alternating
    # ACT/DVE) never sit between rstd ops on the ACT queue.
    xls = {}

    def emit_ln1(nt):
        xt = xp.tile([128, D], BF16, name="xt", tag="xt")
        nc.sync.dma_start(out=xt, in_=x[ts(nt, 128), :])
        if nt == 2:
            dma_tq_block(0, 0)
            dma_tq_block(1, 0)
        if nt == NT - 1:
            # rest of the weight DMAs, queued on sync strictly after x
            for ot in range(1, NP):
                dma_tq_block(0, ot)
                dma_tq_block(1, ot)
            dma_tq_v()
            nc.sync.dma_start(
                out=toT_sb, in_=toT.rearrange("(c p) o -> p c o", p=128)
            )
        st6 = lnp.tile([128, 6], F32, name="st6", tag="st6")
        nc.vector.bn_stats(st6, xt)
        mv = lnp.tile([128, 2], F32, name="mv", tag="mv")
        nc.vector.bn_aggr(mv, st6)
        # rstd = exp(-0.5*ln(var+eps)) — keeps ACT on the ln/exp table set
        sd = lnp.tile([128, 1], F32, name="sd", tag="sd")
        nc.scalar.activation(sd, mv[:, 1:2], AF.Ln, bias=eps1)
        rs = lnp.tile([128, 1], F32, name="rs", tag="rs")
        nc.scalar.activation(rs, sd, AF.Exp, scale=-0.5)
        xl = xlnp.tile([128, D], BF16, name="xl", tag="xl")
        if need_g1 or need_b1:
            xlf = xlnp.tile([128, D], F32, name="xlf", tag="xlf")
            nc.vector.tensor_scalar(
                out=xlf, in0=xt, scalar1=mv[:, 0:1], scalar2=rs,
                op0=ALU.subtract, op1=ALU.mult,
            )
            if need_g1:
                nc.vector.tensor_mul(xlf, xlf, g1b)
            if need_b1:
                nc.vector.tensor_add(xlf, xlf, b1b)
            nc.vector.tensor_copy(xl, xlf)
        else:
            nc.vector.tensor_scalar(
                out=xl, in0=xt, scalar1=mv[:, 0:1], scalar2=rs,
                op0=ALU.subtract, op1=ALU.mult,
            )
        xls[nt] = xl

    def emit_trcopy(nt):
        pt = ps_m.tile([128, DC, 128], F32, name="pt", tag="mm")
        for dc in range(DC):
            nc.tensor.matmul(
                pt[:, dc, :], lhsT=xls[nt][:, ts(dc, 128)], rhs=ident,
                start=True, stop=True,
            )
        nc.scalar.copy(xlnT[:, :, ts(nt, 128)], pt)

    LAG = 3
    for nt in range(NT):
        emit_ln1(nt)
        if nt >= LAG:
            emit_trcopy(nt - LAG)
            if nt - LAG == 3:
                # first query-half of pair-0 q/k (needs xlnT tiles 0..3 only)
                emit_qk_half(0, 0, 0, pool=ps_av)
                emit_qk_half(0, 1, 0, pool=ps_av)
    for nt in range(NT - LAG, NT):
        emit_trcopy(nt)
    emit_qk_half(0, 0, 1, pool=ps_av)
    emit_qk_half(0, 1, 1, pool=ps_av)
    _phase_a[0] = False

    dump("xlnT", xlnT)

    # ================ Phase B helpers ================
    def emit_qk_part(ot, sec):
        """One section (q or k) of head-pair ot: 2 psums of [128, 512]."""
        emit_qk_half(ot, sec, 0)
        emit_qk_half(ot, sec, 1)

    def emit_v(mt):
        # v row-major: psum[m_tile, o] = sum_dc xlnT[:,dc,mt].T @ Tq_v[:,dc,:]
        pv = ps_m.tile([128, 512], F32, name="pv", tag="mm")
        for dc in range(DC):
            nc.tensor.matmul(
                pv,
                lhsT=xlnT[:, dc, ts(mt, 128)],
                rhs=tq_sb[:, dc, ds(2 * INNER, INNER)],
                start=(dc == 0), stop=(dc == DC - 1),
            )
        # strided copy into pair-major layout [pair, sub, 64] with 65-stride
        vv = v_sb[:, mt].rearrange("p pr (s e) -> p pr s e", s=2)[:, :, :, 0:DH]
        nc.vector.tensor_copy(
            out=vv, in_=pv.rearrange("p (pr s d) -> p pr s d", pr=NP, s=2)
        )

    scale_exp = c["scale_exp"]

    def emit_score_sub(p, mt, sub, atns):
        """Scores+exp for one sub-head of pair p, key tile mt."""
        pss = ps_s.tile([128, N], F32, name="pss", tag="s")
        base = sub * 64
        for nn in range(2):
            nc.tensor.matmul(
                pss[:, ts(nn, 512)],
                lhsT=kT[ds(base, 64), p, ts(mt, 128)],
                rhs=qT[ds(base, 64), p, ts(nn, 512)],
                start=True, stop=True,
            )
        nc.scalar.activation(
            out=atns[sub][:, mt, :], in_=pss, func=AF.Exp, scale=scale_exp
        )

    def emit_attnv_sub(p, j, sub, atns, tail=False):
        """attn@v for one sub-head: psum [128, 65], divide, per-sub stats."""
        po = ps_av.tile([128, DH + 1], F32, name="pos", tag="po")
        for mt in range(NT):
            nc.tensor.matmul(
                po,
                lhsT=atns[sub][:, mt, ts(j, 128)],
                rhs=v_sb[:, mt, p, ds(sub * (DH + 1), DH + 1)],
                start=(mt == 0), stop=(mt == NT - 1),
            )
        rc = smp.tile([128, 1], F32, name="rcs", tag="rcs")
        nc.vector.reciprocal(rc, po[:, DH : DH + 1])
        dst = a_sb[:, j, ds(p * 128 + sub * DH, DH)]
        nc.vector.tensor_scalar_mul(dst, po[:, 0:DH], rc)
        nc.vector.bn_stats(
            st_all[:, j, ds(p * 12 + sub * 6, 6)], dst
        )

    def emit_transpose_sub(p, j, sub, act=False):
        """Transpose one sub-head block of chunk j into aT."""
        pt = ps_m.tile([64, 128], F32, name="pts", tag="mm")
        nc.tensor.matmul(
            pt, lhsT=a_sb[:, j, ds(p * 128 + sub * DH, DH)], rhs=ident,
            start=True, stop=True,
        )
        dst = aT[ds(sub * DH, DH), p, ts(j, 128)]
        if act:
            nc.scalar.copy(dst, pt)
        else:
            nc.vector.tensor_copy(out=dst, in_=pt)

    def emit_score_mt(p, mt, atns):
        """Scores+exp for head pair p, key tile mt."""
        pss = [
            ps_s.tile([128, N], F32, name="pssa", tag="s"),
            ps_s.tile([128, N], F32, name="pssb", tag="s"),
        ]
        for nn in range(2):
            for sub in range(2):
                base = sub * 64
                nc.tensor.matmul(
                    pss[sub][:, ts(nn, 512)],
                    lhsT=kT[ds(base, 64), p, ts(mt, 128)],
                    rhs=qT[ds(base, 64), p, ts(nn, 512)],
                    start=True, stop=True,
                )
        for sub in range(2):
            out = atns[sub][:, mt, :]
            if p < NP - 1 and mt in EXP_DVE_MTS and sub == 0:
                nc.vector.tensor_tensor(out=out, in0=ebase, in1=pss[sub], op=ALU.pow)
            elif p < NP - 1 and mt in EXP_POOL_MTS and sub == 1:
                nc.gpsimd.tensor_tensor(out=out, in0=ebase, in1=pss[sub], op=ALU.pow)
            else:
                nc.scalar.activation(
                    out=out, in_=pss[sub], func=AF.Exp, scale=scale_exp
                )

    def emit_attnv_chunk(p, j, atns, tail=False):
        """attn@v for head pair p, query chunk j -> divided rows of a_sb."""
        po = ps_av.tile([128, 2 * (DH + 1)], F32, name="po", tag="po")
        for sub in range(2):
            for mt in range(NT):
                nc.tensor.matmul(
                    po[:, ds(sub * (DH + 1), DH + 1)],
                    lhsT=atns[sub][:, mt, ts(j, 128)],
                    rhs=v_sb[:, mt, p, ds(sub * (DH + 1), DH + 1)],
                    start=(mt == 0), stop=(mt == NT - 1),
                )
        # reciprocal of the two colsum columns (per-partition scalars)
        rc = smp.tile([128, 2, 1], F32, name="rc", tag="rc")
        cs_view = po.rearrange("q (s e) -> q s e", s=2)[:, :, DH : DH + 1]
        nc.vector.reciprocal(rc, cs_view)
        # divide + downcast into row-major a_sb; tail pairs use ACT (idle
        # after the last exp), steady-state pairs split DVE/Pool
        for sub in range(2):
            dst = a_sb[:, j, ds(p * 128 + sub * DH, DH)]
            src = po[:, ds(sub * (DH + 1), DH)]
            if tail:
                nc.scalar.activation(dst, src, AF.Copy, scale=rc[:, sub, :])
            else:
                nc.vector.tensor_scalar_mul(dst, src, rc[:, sub, :])
        # per-pair LN2 stats of this chunk (equal counts -> exact bn_aggr)
        for sub in range(2):
            nc.vector.bn_stats(
                st_all[:, j, ds(p * 12 + sub * 6, 6)],
                a_sb[:, j, ds(p * 128 + sub * DH, DH)],
            )

    def emit_transpose(p, jlo, njs, act=False):
        """Transpose a_sb[:, j, p-block] -> aT[:, p, j*128] for njs chunks."""
        pt = ps_m.tile([128, njs * 128], F32, name="ptr", tag="mm")
        for i in range(njs):
            nc.tensor.matmul(
                pt[:, ts(i, 128)], lhsT=a_sb[:, jlo + i, ds(p * 128, 128)],
                rhs=ident, start=True, stop=True,
            )
        dst = aT[:, p, ds(jlo * 128, njs * 128)]
        if act:
            nc.scalar.copy(dst, pt)
        else:
            nc.vector.tensor_copy(out=dst, in_=pt)

    def emit_ln2_aggr(j):
        nc.vector.bn_aggr(mv_all[:, j, :], st_all[:, j, :])

    def emit_r2_batch(j0, n):
        # r2 = s_o / sqrt(var + eps_eff); r2n = -r2, n chunks at once
        sd2 = smp.tile([128, 4], F32, name="sd2", tag="sd2")
        var_v = mv_all[:, j0 : j0 + n, 1]
        nc.scalar.activation(sd2[:, 0:n], var_v, AF.Ln, bias=eps2, scale=c["inv_so2"])
        r2 = smp.tile([128, 4], F32, name="r2", tag="r2")
        nc.scalar.activation(r2[:, 0:n], sd2[:, 0:n], AF.Exp, scale=-0.5)
        nc.vector.tensor_scalar_mul(r2n_all[:, j0 : j0 + n], r2[:, 0:n], -1.0)

    def emit_zy(j):
        """Output projection + LN2 affine for query chunk j."""
        pz = ps_m.tile([128, INNER], F32, name="pz", tag="mm")
        for ch in range(DC):
            nc.tensor.matmul(
                pz, lhsT=aT[:, ch, ts(j, 128)], rhs=toT_sb[:, ch, :],
                start=(ch == 0), stop=(ch == DC - 1),
            )
        yt = outp.tile([128, INNER], F32, name="yt", tag="yt")
        # u = W1*mu - z  (DVE: reads PSUM);  y = u * (-r2) split ACT/DVE so
        # no single engine paces the tail
        nc.vector.scalar_tensor_tensor(
            out=yt, in0=w1b, scalar=mv_all[:, j, 0:1], in1=pz,
            op0=ALU.mult, op1=ALU.subtract,
        )
        if j >= NT - 2:
            nc.scalar.mul(yt, yt, r2n_all[:, j : j + 1])
        elif YMUL_POOL:
            nc.gpsimd.tensor_scalar_mul(yt, yt, r2n_all[:, j : j + 1])
        else:
            nc.vector.tensor_scalar_mul(yt, yt, r2n_all[:, j : j + 1])
        if need_bt:
            nc.vector.tensor_add(yt, yt, btb)
        nc.sync.dma_start(out=y[ts(j, 128), :], in_=yt)

    # ================ Phase C driver ================
    # Pair p's scores stream per key-tile mt; exp (ACT) is the pacer. PE
    # fill-work (qk of the next pair, v, attn@v + transposes of the previous
    # pair) is emitted BEFORE each score tile so it runs in the stall gaps.
    atn_pairs = []

    def new_atn_tiles():
        return [
            attp.tile([128, NT, N], BF16, name=f"atn{s}", tag=f"atn{s}")
            for s in range(2)
        ]

    # fillers[p] = list of thunks to interleave into pair p's 8 mt slots.
    # attn@v of the previous pair comes FIRST so its atn tiles free early
    # (the attp pool has 2 slots; pair p+1's exps wait on pair p-1's attnv).
    fillers = {p: [] for p in range(NP)}
    fillers[0] = [
        lambda: emit_qk_part(1, 0), lambda: emit_qk_part(1, 1),
    ] + [lambda mt=mt: emit_v(mt) for mt in range(6)]
    fillers[1] = [lambda mt=mt: emit_v(mt) for mt in (6, 7)]
    tail_fill = {
        1: [lambda: emit_qk_part(2, 0), lambda: emit_qk_part(2, 1)],
        2: [lambda: emit_qk_part(3, 0), lambda: emit_qk_part(3, 1)],
    }

    def run_pair(p, atns, fl, scores):
        """Emit the 8 score units of `scores` with fillers spread between."""
        per = (len(fl) + NT - 1) // NT if fl else 0
        fi = 0
        for i, sc in enumerate(scores):
            if not (p == 0 and i == 0):
                take = min(per, len(fl) - fi) if fi < len(fl) else 0
                if i == len(scores) - 1:
                    take = len(fl) - fi
                for _ in range(take):
                    fl[fi]()
                    fi += 1
            sc()

    for p in range(NP):
        atns = new_atn_tiles()
        atn_pairs.append(atns)
        fl = list(fillers[p])
        if p >= 1:
            prev = atn_pairs[p - 1]
            av = [
                (lambda p1=p - 1, j=j, a=prev: emit_attnv_chunk(p1, j, a))
                for j in range(NT)
            ]
            fl = fl + av[:4] + tail_fill.get(p, []) + av[4:] + [
                (lambda p1=p - 1: emit_transpose(p1, 0, 4)),
                (lambda p1=p - 1: emit_transpose(p1, 4, 4)),
            ]
        else:
            fl = fl + tail_fill.get(p, [])
        if p < NP - 1:
            run_pair(p, atns, fl,
                     [lambda mt=mt: emit_score_mt(p, mt, atns) for mt in range(NT)])
        else:
            # last pair: all sub-0 exps first, then sub-1; sub-0's attn@v,
            # divides, stats and transposes overlap sub-1's 8.3us exp window
            run_pair(p, atns, fl,
                     [lambda mt=mt: emit_score_sub(p, mt, 0, atns) for mt in range(NT)])
            fl2 = []
            for j in range(NT):
                fl2.append(lambda j=j: emit_attnv_sub(p, j, 0, atns))
            for j in range(NT):
                fl2.append(lambda j=j: emit_transpose_sub(p, j, 0))
            run_pair(p, atns, fl2,
                     [lambda mt=mt: emit_score_sub(p, mt, 1, atns) for mt in range(NT)])

    # ================ tail: last pair's attn@v + LN2 + projection ================
    last = atn_pairs[NP - 1]
    for j in range(NT):
        emit_attnv_sub(NP - 1, j, 1, last, tail=True)
        if j >= 2:
            emit_ln2_aggr(j - 2)
            emit_r2_batch(j - 2, 1)
            emit_transpose_sub(NP - 1, j - 2, 1, act=True)
        if j in (4, 5):
            emit_zy(j - 4)
    for j in (NT - 2, NT - 1):
        emit_ln2_aggr(j)
        emit_r2_batch(j, 1)
        emit_transpose_sub(NP - 1, j, 1, act=True)
    for j in range(2, NT):
        emit_zy(j)

    dump("qT", qT)
    dump("kT", kT)
    dump("v", v_sb)
    dump("a_sb", a_sb)
    dump("aT", aT)


def _build(c: dict):
    nc = bacc.Bacc("TRN2", target_bir_lowering=False, debug=False, num_devices=B)
    io = {
        "x": nc.dram_tensor("x", [N, D], BF16, kind="ExternalInput").ap(),
        "tqT": nc.dram_tensor("tqT", [D, 3 * INNER], BF16, kind="ExternalInput").ap(),
        "toT": nc.dram_tensor("toT", [INNER, INNER], BF16, kind="ExternalInput").ap(),
        "w1u": nc.dram_tensor("w1u", [INNER], F32, kind="ExternalInput").ap(),
        "y": nc.dram_tensor("y", [N, D], F32, kind="ExternalOutput").ap(),
    }
    if c["need_g1"]:
        io["g1v"] = nc.dram_tensor("g1v", [D], F32, kind="ExternalInput").ap()
    if c["need_b1"]:
        io["b1v"] = nc.dram_tensor("b1v", [D], F32, kind="ExternalInput").ap()
    if c["need_bt"]:
        io["btv"] = nc.dram_tensor("btv", [INNER], F32, kind="ExternalInput").ap()
    reps = c.get("body_reps", 1)
    with tile.TileContext(nc) as tc:
        for r in range(reps):
            with ExitStack() as ctx:
                _emit(ctx, tc, io, c, sfx="" if r == 0 else f"_r{r}")

    nc.compile()

    # The act-table-load pass greedily picks the first set containing each
    # function, thrashing between `natural_log` (Ln) and `exp_and_others`
    # (Exp). All activation funcs this kernel uses (Ln, Exp, Copy, Identity)
    # live together in `natural_log_exp_and_others`, so rewrite the first
    # load to that set and drop the rest.
    from concourse.hw_specs import get_activation_tables
    tset = list(get_activation_tables(nc.m.arch).keys())
    nle = tset.index("natural_log_exp_and_others")
    for blk in nc.main_func.blocks:
        keep, first = [], False
        for inst in blk.instructions:
            if type(inst).__name__ == "InstLoadActFuncSet":
                si = getattr(inst, "sync_info", None)
                clean = si is None or (not si.on_wait and not si.on_update)
                if not first:
                    inst.act_func_set_id = nle
                    first = True
                    keep.append(inst)
                elif not clean:
                    inst.act_func_set_id = nle
                    keep.append(inst)
            else:
                keep.append(inst)
        blk.instructions[:] = keep
    return nc


def _prep(inputs):
    g1 = np.asarray(inputs["g1"], np.float32)
    b1 = np.asarray(inputs["b1"], np.float32)
    g2 = np.asarray(inputs["g2"], np.float32)
    b2 = np.asarray(inputs["b2"], np.float32)
    b_out = np.asarray(inputs["b_out"], np.float32)

    Tq, s_q = _ternary(inputs["W_qkv"])   # [3*inner, d]
    To, s_o = _ternary(inputs["W_out"])   # [dout, o]

    Wp = To * g2[None, :]                 # fold g2 (exact when g2 == 1)
    toT = np.ascontiguousarray(Wp.T)      # [o, dout]
    w1u = Wp.sum(axis=1).astype(np.float32)
    bias_total = (b2 @ To.T) * np.float32(s_o) + b_out

    c = {
        "scale_exp": float(s_q * s_q * (DH ** -0.5)),
        "inv_so2": float(1.0 / (s_o * s_o)),
        "eps_eff": float(EPS_LN / (s_q * s_q * s_o * s_o)),
        "need_g1": bool(not np.allclose(g1, 1.0)),
        "need_b1": bool(np.any(b1)),
        "need_bt": bool(np.any(bias_total)),
    }
    arrs = {
        "tqT": np.ascontiguousarray(Tq.T),
        "toT": toT,
        "w1u": w1u,
        "g1": g1, "b1": b1, "bt": bias_total,
    }
    return c, arrs


def _to_bf16(a):
    import ml_dtypes
    return np.asarray(a, np.float32).astype(ml_dtypes.bfloat16)


def kernel(**inputs) -> np.ndarray:
    global LAST_RESULTS
    x = np.asarray(inputs["x"], np.float32)
    assert x.shape == (B, N, D)
    c, arrs = _prep(inputs)

    key = tuple(sorted(c.items()))
    if key not in _CACHE:
        _CACHE[key] = _build(c)
    nc = _CACHE[key]

    base = {
        "tqT": _to_bf16(arrs["tqT"]),
        "toT": _to_bf16(arrs["toT"]),
        "w1u": arrs["w1u"].astype(np.float32),
    }
    if c["need_g1"]:
        base["g1v"] = arrs["g1"]
    if c["need_b1"]:
        base["b1v"] = arrs["b1"]
    if c["need_bt"]:
        base["btv"] = arrs["bt"].astype(np.float32)

    in_maps = [dict(base, x=_to_bf16(np.ascontiguousarray(x[i]))) for i in range(B)]
    res = run_bass_kernel_spmd(nc, in_maps, core_ids=list(range(B)), trace=TRACE)
    LAST_RESULTS = res
    out = np.stack([res.results[i]["y"] for i in range(B)], axis=0)
    return out.astype(np.float32)


def bench_exec_ns(inputs, iters=32, reps=5, body_reps=1):
    """Measure per-execution NEFF time by chaining `iters` sequential
    executions inside one jitted program (chained through the output
    buffers) and comparing against a 1-execution program."""
    import time as _time
    import jax
    from jax.experimental.shard_map import shard_map
    from jax.sharding import Mesh, PartitionSpec, NamedSharding
    from concourse import bass2jax, mybir as _mybir

    x = np.asarray(inputs["x"], np.float32)
    c, arrs = _prep(inputs)
    if body_reps != 1:
        c["body_reps"] = body_reps
    key = tuple(sorted(c.items()))
    if key not in _CACHE:
        _CACHE[key] = _build(c)
    nc = _CACHE[key]
    bass2jax.install_neuronx_cc_hook()

    base = {
        "tqT": _to_bf16(arrs["tqT"]),
        "toT": _to_bf16(arrs["toT"]),
        "w1u": arrs["w1u"].astype(np.float32),
    }
    if c["need_g1"]:
        base["g1v"] = arrs["g1"]
    if c["need_b1"]:
        base["b1v"] = arrs["b1"]
    if c["need_bt"]:
        base["btv"] = arrs["bt"].astype(np.float32)
    in_maps = [dict(base, x=_to_bf16(np.ascontiguousarray(x[i]))) for i in range(B)]

    partition_name = nc.partition_id_tensor.name if nc.partition_id_tensor else None
    in_names, out_names, out_avals, zero_outs = [], [], [], []
    for alloc in nc.m.functions[0].allocations:
        if not isinstance(alloc, mybir.MemoryLocationSet):
            continue
        name = alloc.memorylocations[0].name
        if alloc.kind == "ExternalInput":
            if name != partition_name:
                in_names.append(name)
        elif alloc.kind == "ExternalOutput":
            out_names.append(name)
            shape = tuple(alloc.tensor_shape)
            dtype = mybir.dt.np(alloc.dtype)
            out_avals.append(jax.core.ShapedArray(shape, dtype))
            zero_outs.append(np.zeros(shape, dtype))
    n_params = len(in_names)

    bind_names = list(in_names) + list(out_names)
    if partition_name is not None:
        bind_names.append(partition_name)

    def _body(*args):
        operands = list(args)
        pid = [bass2jax.partition_id_tensor()] if partition_name else []
        outs = bass2jax._bass_exec_p.bind(
            *(operands + pid),
            out_avals=tuple(out_avals),
            in_names=tuple(bind_names),
            out_names=tuple(out_names),
            lowering_input_output_aliases=(),
            sim_require_finite=True,
            sim_require_nnan=True,
            nc=nc,
        )
        return tuple(outs)

    devices = jax.devices()[:B]
    mesh = Mesh(np.asarray(devices), ("core",))
    spec = PartitionSpec("core")
    n_out = len(out_names)
    per_core = [[np.asarray(m[nm]) for nm in in_names] for m in in_maps]
    concat_in = [
        np.concatenate([per_core[cc][i] for cc in range(B)], axis=0)
        for i in range(n_params)
    ]
    concat_zeros = [
        np.zeros((B * z.shape[0], *z.shape[1:]), z.dtype) for z in zero_outs
    ]
    dev_args = [
        jax.device_put(a, NamedSharding(mesh, spec)) for a in concat_in + concat_zeros
    ]

    f = jax.jit(
        shard_map(
            _body, mesh=mesh,
            in_specs=(spec,) * (n_params + n_out),
            out_specs=(spec,) * n_out,
            check_rep=False,
        )
    )
    jax.block_until_ready(f(*dev_args))  # compile + warm

    times = {}
    for k in (1, iters):
        best = float("inf")
        for _ in range(reps):
            t0 = _time.perf_counter()
            r = None
            for _ in range(k):
                r = f(*dev_args)  # async dispatch; device executes in-order
            jax.block_until_ready(r)
            best = min(best, _time.perf_counter() - t0)
        times[k] = best
    exec_ns = (times[iters] - times[1]) / (iters - 1) * 1e9
    return exec_ns, times
